# revision 27
# baseline (speedup 1.0000x reference)
"""Trainium2 Bass kernel for the decoupled sparse-attention layer.

Sharding: 8 cores = 2 batch x 4 GQA head-groups. Core i handles batch
b=i//4 and query heads [4g..4g+4) with KV head g, g=i%4. Each core
computes a partial output y_partial = attn_heads @ Wo_rows(group); the
host sums the 4 group partials per batch element.

Schedule (v3 — fused pipeline):
  Phase 1: projection GEMM stream for all 8 t-chunks + RoPE + incremental
    KV pooling, with the attention for query chunks 0..5 (which only see
    pooled mem-block keys) fused into the stream: attention for chunk c
    is issued as closures popped between the projection matmuls of chunk
    c+2, so its engine work (exp/mask/normalize) hides under the
    PE-dense projection stream. The pooled-V transpose re-transposes the
    whole prefix each chunk (same PE cost; keeps the v2 write at
    partition 0).
  Tail: attention for chunks 6,7 (the key-heavy ones) with all 8 chunks'
    out-projection matmuls interleaved as PE fillers. Softmax
    reciprocals via DVE reciprocal_approx_fast on an SBUF-staged
    denominator row; normalization is fused into the PSUM read with one
    scalar_tensor_tensor per head. PSUM->SBUF output casts alternate
    between ACT and DVE.

Per-core layouts (feature dim on partitions):
  xT      [2048, 4096] bf16 input activations (host pre-transposes)
  W_all   [2048, 384] bf16 fused projection weights, output cols:
            [0:128)   q_sem 4 heads x 32, scaled by exp(ls_h)/sqrt(32)
            [128:256) q_geo head-major [x1(16)|x2(16)] per head, scaled
            [256:288) k_sem 32
            [288:320) k_geo [x1(16)|x2(16)]
            [320:384) v 64
  q01_all/q23_all [128, T] bf16: per 64-row head slot [sem 32|x1' 16|x2' 16]
  Keys: 1152 padded slots = [48 mem-blocks | 80 pad | 1024 local].
"""

import numpy as np

B, T, D = 2, 4096, 2048
H, HKV, DS, DG, DV = 16, 4, 32, 32, 64
MB, LW = 64, 1024
REMOTE = T - LW            # 3072
NBLK = REMOTE // MB        # 48
NKEY = NBLK + LW           # 1072
KPAD = 128 + LW            # 1152 padded key slots
NKT = KPAD // 128          # 9 key tiles
TC = 512                   # t-chunk size
NC_CHUNKS = T // TC        # 8
ROPE_BASE = 10000.0

_PROG = None

SWAP16 = list(range(16, 32)) + list(range(0, 16))

# per-chunk count (of 16 outproj casts) assigned to the scalar engine
ACT_CAST_SHARE = [5, 5, 5, 5, 5, 5, 5, 8]


def _active_tiles(c):
    """Key tiles (tile_idx, nrows) visible to query chunk c, plus which
    tiles need the mask path."""
    tiles = [(0, 8 * (c + 1))] if c <= 5 else [(0, NBLK)]
    if c >= 6:
        nloc = (c - 5) * TC
        for t in range(1, 1 + nloc // 128):
            tiles.append((t, 128))
    masked = set()
    if c <= 5:
        masked.add(0)
    else:
        for t, n in tiles[1:]:
            maxpos = REMOTE + t * 128 - 1
            if maxpos > 512 * c:
                masked.add(t)
    return tiles, masked


def _build_program():
    from contextlib import ExitStack
    import concourse.bass as bass
    import concourse.bacc as bacc
    import concourse.tile as tile
    from concourse import mybir

    f32 = mybir.dt.float32
    bf16 = mybir.dt.bfloat16
    Alu = mybir.AluOpType
    Act = mybir.ActivationFunctionType

    nc = bacc.Bacc()
    xT = nc.declare_dram_parameter("xT", [D, T], bf16, isOutput=False)
    W_all = nc.declare_dram_parameter("W_all", [D, 384], bf16, isOutput=False)
    Wo = nc.declare_dram_parameter("Wo", [256, D], bf16, isOutput=False)
    c32d = nc.declare_dram_parameter("c32d", [32, T], bf16, isOutput=False)
    s32d = nc.declare_dram_parameter("s32d", [32, T], bf16, isOutput=False)
    kpos = nc.declare_dram_parameter("kpos", [KPAD], f32, isOutput=False)
    qpos = nc.declare_dram_parameter("qpos", [T], f32, isOutput=False)
    ident = nc.declare_dram_parameter("ident", [64, 64], f32, isOutput=False)
    y = nc.declare_dram_parameter("y", [T, D], bf16, isOutput=True)

    with tile.TileContext(nc) as tc, ExitStack() as ctx:
        persist = ctx.enter_context(tc.tile_pool(name="persist", bufs=1))
        xpool = ctx.enter_context(tc.tile_pool(name="x", bufs=3))
        tmp = ctx.enter_context(tc.tile_pool(name="tmp", bufs=2))
        epool = ctx.enter_context(tc.tile_pool(name="e", bufs=4))
        mpool = ctx.enter_context(tc.tile_pool(name="m", bufs=4))
        ypool = ctx.enter_context(tc.tile_pool(name="y", bufs=3))
        npool = ctx.enter_context(tc.tile_pool(name="n", bufs=2))

        # ---- persistent SBUF tensors ----
        wall_sb = persist.tile([128, 16, 384], bf16)

        def wall_dma(wh):
            nc.gpsimd.dma_start(
                out=wall_sb[:, 4 * wh:4 * wh + 4, :],
                in_=bass.AP(tensor=W_all, offset=wh * 4 * 128 * 384,
                            ap=[[384, 128], [384 * 128, 4], [1, 384]]))

        wo_sb = persist.tile([128, 2, D], bf16)
        C128 = persist.tile([128, T], bf16)
        S128 = persist.tile([128, T], bf16)
        ident_sb = persist.tile([64, 64], f32)
        nc.scalar.dma_start(out=ident_sb, in_=ident[:, :])
        kpos_sb = persist.tile([128, NKT], f32)
        nc.scalar.dma_start(
            out=kpos_sb,
            in_=bass.AP(tensor=kpos, offset=0, ap=[[1, 128], [128, NKT]]))
        qpos_all = persist.tile([128, T], f32)

        q01_all = persist.tile([128, T], bf16)
        q23_all = persist.tile([128, T], bf16)
        aT01 = persist.tile([128, T], bf16)
        aT23 = persist.tile([128, T], bf16)
        kTd = persist.tile([128, KPAD], bf16)   # [sem32|x1'16|x2'16] dup'd
        vT = persist.tile([64, KPAD], f32)
        nc.vector.memset(vT, 0.0)
        v2 = persist.tile([128, NKT, 65], bf16)  # [key, dv | ones]
        onesrc = persist.tile([128, 1], f32)
        nc.vector.memset(onesrc, 1.0)
        nc.vector.tensor_copy(out=v2[0:NBLK, 0, 64:65], in_=onesrc[0:NBLK, :])
        for t in range(1, NKT):
            nc.vector.tensor_copy(out=v2[:, t, 64:65], in_=onesrc)
        vsum = persist.tile([64, 1], f32)

        qsrc = [q01_all, q01_all, q23_all, q23_all]
        qb4 = [0, 64, 0, 64]          # 64-row slot base per head
        aTs = [aT01, aT01, aT23, aT23]

        def rel_unit(c, h, outp_h, recs):
            """Free an outp bank fast: den stage + unnormalized aT cast on
            ACT, reciprocal on DVE."""
            sl = slice(c * TC, (c + 1) * TC)
            hh = h % 2
            den = npool.tile([1, TC], f32, tag=f"den{hh}", bufs=1,
                             name=f"den_{c}_{h}")
            nc.scalar.copy(out=den, in_=outp_h[64:65, :])
            base = qb4[h]
            nc.scalar.copy(out=aTs[h][base:base + 64, sl],
                           in_=outp_h[0:64, :])
            rec = npool.tile([1, TC], f32, tag=f"rec{hh}", bufs=1,
                             name=f"rec_{c}_{h}")
            nc.vector.reciprocal_approx_fast(out=rec, in_=den)
            recs[h] = rec

        def norm_pair(c, pair, recs):
            """Trailing normalize: assemble the pair's [128,TC] reciprocal
            broadcast (both TT inputs must share base partition 0) and one
            in-place multiply."""
            sl = slice(c * TC, (c + 1) * TC)
            rbP = npool.tile([128, TC], f32, tag="rbP", bufs=2,
                             name=f"rbP_{c}_{pair[0]}")
            rbt = npool.tile([64, TC], f32, tag="rbt", bufs=2,
                             name=f"rbt_{c}_{pair[0]}")
            nc.gpsimd.partition_broadcast(out_ap=rbP[0:64, :],
                                          in_ap=recs[pair[0]])
            nc.gpsimd.partition_broadcast(out_ap=rbt, in_ap=recs[pair[1]])
            nc.vector.tensor_copy(out=rbP[64:128, :], in_=rbt)
            dst = aT01 if pair[0] < 2 else aT23
            nc.vector.tensor_mul(dst[:, sl], dst[:, sl], rbP)

        # ========== PHASE 1: projections + fused attention (chunks 0-5) ====
        with tc.tile_pool(name="psp", bufs=2, space="PSUM") as ps_proj, \
             tc.tile_pool(name="psatt", bufs=1, space="PSUM") as ps_att:

            def fused_attn_closures(cq):
                """Attention for query chunk cq (pooled keys only), issued
                as closures popped between chunk cq+2's projection matmuls."""
                lo = cq * TC
                sl = slice(lo, lo + TC)
                n = 8 * (cq + 1)
                m_sb = mpool.tile([128, TC], bf16, tag="mask", bufs=6,
                                  name=f"mf{cq}")
                nc.vector.tensor_scalar(
                    out=m_sb[0:n, :], in0=qpos_all[0:n, sl],
                    scalar1=kpos_sb[0:n, 0:1], scalar2=None, op0=Alu.is_ge)
                es = {}
                outp = {}

                def vtr():
                    pv = ps_att.tile([128, 64], f32, tag="sc", name=f"pvf{cq}")
                    nc.tensor.transpose(out=pv[0:n, :], in_=vT[:, 0:n],
                                        identity=ident_sb)
                    nc.vector.tensor_copy(out=v2[0:n, 0, 0:64], in_=pv[0:n, :])

                def mk_sc(h):
                    def f():
                        qb = qb4[h]
                        sc = ps_att.tile([128, TC], f32, tag="sc",
                                         name=f"scf{cq}_{h}")
                        nc.tensor.matmul(out=sc[0:n, :],
                                         lhsT=kTd[qb:qb + 64, 0:n],
                                         rhs=qsrc[h][qb:qb + 64, sl],
                                         start=True, stop=True)
                        e_sb = epool.tile([128, TC], bf16, tag="e",
                                          name=f"ef{cq}_{h}")
                        nc.scalar.activation(out=e_sb[0:n, :], in_=sc[0:n, :],
                                             func=Act.Exp)
                        nc.vector.tensor_mul(e_sb[0:n, :], e_sb[0:n, :],
                                             m_sb[0:n, :])
                        es[h] = e_sb
                    return f

                def mk_pv(h):
                    def f():
                        outp[h] = ps_att.tile([65, TC], f32, tag="outp",
                                              name=f"outpf{cq}_{h}")
                        nc.tensor.matmul(out=outp[h], lhsT=v2[0:n, 0, :],
                                         rhs=es[h][0:n, :],
                                         start=True, stop=True)
                    return f

                recs = {}

                def mk_fin(h):
                    return lambda: rel_unit(cq, h, outp[h], recs)

                def mk_norm(pair):
                    return lambda: norm_pair(cq, pair, recs)

                return [vtr, mk_sc(0), mk_pv(0), mk_sc(1), mk_fin(0),
                        mk_pv(1), mk_sc(2), mk_fin(1), mk_norm((0, 1)),
                        mk_pv(2), mk_sc(3), mk_fin(2), mk_pv(3), mk_fin(3),
                        mk_norm((2, 3))]

            pend, nextp = [], []
            xts = {}
            for c in range(NC_CHUNKS):
                lo = c * TC
                sl = slice(lo, lo + TC)
                xt = xts[c] if c == 1 else xpool.tile([128, 16, TC], bf16,
                                                       tag="xt")
                if c == 0:
                    # interleave x and W pieces so the first matmul's inputs
                    # land first (startup is HBM-bandwidth-bound)
                    def xt_dma(kq):
                        nc.sync.dma_start(
                            out=xt[:, 4 * kq:4 * kq + 4, :],
                            in_=bass.AP(tensor=xT, offset=kq * 4 * 128 * T + lo,
                                        ap=[[T, 128], [T * 128, 4], [1, TC]]))
                    xt_dma(0); wall_dma(0); wall_dma(1); xt_dma(1)
                    wall_dma(2); wall_dma(3); xt_dma(2); xt_dma(3)
                    # prefetch chunk 1's x so the c0->c1 boundary has DMA lead
                    xts[1] = xpool.tile([128, 16, TC], bf16, tag="xt",
                                        name="xt1pf")
                    for xh in range(2):
                        nc.scalar.dma_start(
                            out=xts[1][:, 8 * xh:8 * xh + 8, :],
                            in_=bass.AP(tensor=xT,
                                        offset=xh * 8 * 128 * T + TC,
                                        ap=[[T, 128], [T * 128, 8], [1, TC]]))
                    # cos/sin: one HBM load + on-chip replication (SB->SB DMA)
                    nc.scalar.dma_start(out=C128[0:32, :], in_=c32d[:, :])
                    nc.scalar.dma_start(out=S128[0:32, :], in_=s32d[:, :])
                    for qd in range(1, 4):
                        nc.scalar.dma_start(out=C128[32 * qd:32 * qd + 32, :],
                                            in_=C128[0:32, :])
                        nc.scalar.dma_start(out=S128[32 * qd:32 * qd + 32, :],
                                            in_=S128[0:32, :])
                elif c == 1:
                    nc.scalar.dma_start(
                        out=wo_sb,
                        in_=bass.AP(tensor=Wo, offset=0,
                                    ap=[[D, 128], [D * 128, 2], [1, D]]))
                else:
                    for xh in range(2):
                        nc.sync.dma_start(
                            out=xt[:, 8 * xh:8 * xh + 8, :],
                            in_=bass.AP(tensor=xT,
                                        offset=xh * 8 * 128 * T + lo,
                                        ap=[[T, 128], [T * 128, 8], [1, TC]]))
                if c <= 3:
                    # qpos pieces 2c, 2c+1 (needed by fused masks from chunk c)
                    for qq in range(2):
                        qlo = (2 * c + qq) * TC
                        nc.scalar.dma_start(
                            out=qpos_all[:, qlo:qlo + TC],
                            in_=bass.AP(tensor=qpos, offset=qlo,
                                        ap=[[0, 128], [1, TC]]))
                psA = ps_proj.tile([128, TC], f32, tag="psA")
                psB = ps_proj.tile([128, TC], f32, tag="psB")
                psC = ps_proj.tile([128, TC], f32, tag="psC")
                for kk in range(16):
                    st, sp = kk == 0, kk == 15
                    w = wall_sb[:, kk, :]
                    xk = xt[:, kk, :]
                    nc.tensor.matmul(out=psA, lhsT=w[:, 0:128], rhs=xk,
                                     start=st, stop=sp)
                    nc.tensor.matmul(out=psB, lhsT=w[:, 128:256], rhs=xk,
                                     start=st, stop=sp)
                    nc.tensor.matmul(out=psC, lhsT=w[:, 256:384], rhs=xk,
                                     start=st, stop=sp)
                    if pend:
                        pend.pop(0)()

                # q_sem copies into the 64-row head slots (ACT, bf16 out)
                nc.scalar.copy(out=q01_all[0:32, sl], in_=psA[0:32, :])
                nc.scalar.copy(out=q01_all[64:96, sl], in_=psA[32:64, :])
                nc.scalar.copy(out=q23_all[0:32, sl], in_=psA[64:96, :])
                nc.scalar.copy(out=q23_all[64:96, sl], in_=psA[96:128, :])
                # q_geo RoPE for all 4 heads on DVE
                swq = tmp.tile([128, TC], f32, tag="swq")
                t1q = tmp.tile([128, TC], bf16, tag="t1q")
                sw2 = tmp.tile([128, TC], bf16, tag="sw2")
                nc.vector.stream_shuffle(out=swq, in_=psB, mask=SWAP16)
                nc.vector.tensor_mul(t1q, psB, C128[:, sl])
                nc.vector.tensor_mul(sw2, swq, S128[:, sl])
                for h in range(4):
                    nc.vector.tensor_add(
                        qsrc[h][qb4[h] + 32:qb4[h] + 64, sl],
                        t1q[32 * h:32 * h + 32, :], sw2[32 * h:32 * h + 32, :])

                # k side: rope geo, then pool (c<=5) or copy local (c>=6)
                swp = tmp.tile([64, TC], f32, tag="swp")
                t1 = tmp.tile([32, TC], bf16, tag="t1")
                t2 = tmp.tile([32, TC], bf16, tag="t2")
                blk = psC[32:64, :]
                nc.vector.stream_shuffle(out=swp[32:64, :], in_=blk, mask=SWAP16)
                nc.vector.tensor_mul(t1, blk, C128[0:32, sl])
                nc.vector.tensor_mul(t2, swp[32:64, :], S128[32:64, sl])
                if c <= 5:
                    ktmp = tmp.tile([64, TC], bf16, tag="ktmp")
                    nc.scalar.copy(out=ktmp[0:32, :], in_=psC[0:32, :])
                    nc.vector.tensor_add(ktmp[32:64, :], t1, t2)
                    bs = slice(c * 8, (c + 1) * 8)
                    with nc.allow_low_precision(reason="bf16 pooled keys"):
                        nc.vector.tensor_reduce(
                            out=kTd[0:64, bs],
                            in_=ktmp.rearrange("p (n w) -> p n w", w=MB),
                            axis=mybir.AxisListType.X, op=Alu.add)
                    nc.vector.tensor_scalar_mul(kTd[0:64, bs], kTd[0:64, bs], 1.0 / MB)
                    nc.vector.tensor_reduce(
                        out=vT[:, bs],
                        in_=psC[64:128, :].rearrange("p (n w) -> p n w", w=MB),
                        axis=mybir.AxisListType.X, op=Alu.add)
                    nc.vector.tensor_scalar_mul(vT[:, bs], vT[:, bs], 1.0 / MB)
                    # incremental kTd dup for the new pooled cols
                    nc.scalar.copy(out=kTd[64:128, bs], in_=kTd[0:64, bs])
                else:
                    loff = 128 + (c - 6) * TC
                    lsl = slice(loff, loff + TC)
                    nc.scalar.copy(out=kTd[0:32, lsl], in_=psC[0:32, :])
                    nc.vector.tensor_add(kTd[32:64, lsl], t1, t2)
                    nc.scalar.copy(out=vT[:, lsl], in_=psC[64:128, :])

                while pend:
                    pend.pop(0)()
                pend = nextp
                nextp = fused_attn_closures(c) if c <= 5 else []
            while pend:
                pend.pop(0)()

        # ===== TAIL: attention chunks 6,7 + all out-projections =====
        with tc.tile_pool(name="pssc", bufs=2, space="PSUM") as ps_sc, \
             tc.tile_pool(name="psout", bufs=2, space="PSUM") as ps_out, \
             tc.tile_pool(name="psy", bufs=4, space="PSUM") as ps_y:

            # local kTd dup (cols 128..1152) — needed by chunks 6,7
            nc.scalar.copy(out=kTd[64:128, 128:KPAD], in_=kTd[0:64, 128:KPAD])
            # vsum + uniform-row patch for chunk 0 (queries with no visible key)
            nc.vector.tensor_reduce(out=vsum, in_=vT, axis=mybir.AxisListType.X,
                                    op=Alu.add)
            nc.vector.tensor_scalar_mul(vsum, vsum, 1.0 / float(NKEY))
            for dst in (aT01, aT23):
                for base in (0, 64):
                    nc.vector.tensor_copy(
                        out=dst[base:base + 64, 0:63],
                        in_=vsum.broadcast_to([64, 63]))

            # local V transposes into v2 — issued as early fillers
            def v2_unit(i):
                pv = ps_sc.tile([128, 64], f32, tag="sc", name=f"pv{i}")
                nc.tensor.transpose(out=pv,
                                    in_=vT[:, 128 + 128 * i:256 + 128 * i],
                                    identity=ident_sb)
                nc.vector.tensor_copy(out=v2[:, 1 + i, 0:64], in_=pv)

            ysb = {}
            cast_ctr = [0, 0]   # [chunk-local cast idx, chunk]

            def outproj_unit(tt, nn):
                tsl = slice(tt * 128, (tt + 1) * 128)
                nsl = slice(nn * 512, (nn + 1) * 512)
                yp = ps_y.tile([128, 512], f32, tag="yp")
                nc.tensor.matmul(out=yp, lhsT=aT01[:, tsl],
                                 rhs=wo_sb[:, 0, nsl], start=True, stop=False)
                nc.tensor.matmul(out=yp, lhsT=aT23[:, tsl],
                                 rhs=wo_sb[:, 1, nsl], start=False, stop=True)
                if tt not in ysb:
                    ysb[tt] = ypool.tile([128, D], bf16, tag="ysb",
                                         name=f"ysb{tt}")
                cc = tt // 4
                if cast_ctr[1] != cc:
                    cast_ctr[0], cast_ctr[1] = 0, cc
                on_act = cast_ctr[0] < ACT_CAST_SHARE[cc]
                cast_ctr[0] += 1
                if on_act:
                    nc.scalar.copy(out=ysb[tt][:, nsl], in_=yp)
                else:
                    nc.vector.tensor_copy(out=ysb[tt][:, nsl], in_=yp)
                if nn == 3:
                    nc.sync.dma_start(out=y[tsl, :], in_=ysb.pop(tt))

            fillers = [(lambda i=i: v2_unit(i)) for i in range(8)]
            fillers += [(lambda tt=tt, nn=nn: outproj_unit(tt, nn))
                        for tt in range(24) for nn in range(4)]
            for _ in range(12):
                fillers.pop(0)()

            for c in (6, 7):
                lo = c * TC
                sl = slice(lo, lo + TC)
                tiles, masked = _active_tiles(c)
                mdict = {}
                for (mt, n) in tiles:
                    if mt in masked:
                        m_sb = mpool.tile([128, TC], bf16, tag="mask", bufs=6)
                        nc.vector.tensor_scalar(
                            out=m_sb[0:n, :], in0=qpos_all[0:n, sl],
                            scalar1=kpos_sb[0:n, mt:mt + 1], scalar2=None,
                            op0=Alu.is_ge)
                        mdict[mt] = m_sb

                outp = {}
                last_ti = len(tiles) - 1

                for pair in ((0, 1), (2, 3)):
                    for h in pair:
                        outp[h] = ps_out.tile([65, TC], f32, tag="out",
                                              name=f"outp{c}_{h}")
                    for ti, (kt, n) in enumerate(tiles):
                        ks = slice(kt * 128, kt * 128 + n)
                        es = {}
                        for h in pair:
                            qb = qb4[h]
                            sc = ps_sc.tile([128, TC], f32, tag="sc",
                                            name=f"sc{c}_{h}_{kt}")
                            nc.tensor.matmul(out=sc[0:n, :],
                                             lhsT=kTd[qb:qb + 64, ks],
                                             rhs=qsrc[h][qb:qb + 64, sl],
                                             start=True, stop=True)
                            e_sb = epool.tile([128, TC], bf16, tag="e",
                                              name=f"e{c}_{h}_{kt}")
                            nc.scalar.activation(out=e_sb[0:n, :],
                                                 in_=sc[0:n, :], func=Act.Exp)
                            if kt in mdict:
                                nc.vector.tensor_mul(
                                    e_sb[0:n, :], e_sb[0:n, :],
                                    mdict[kt][0:n, :])
                            es[h] = e_sb
                        for _ in range(6):
                            if fillers:
                                fillers.pop(0)()
                        for h in pair:
                            nc.tensor.matmul(out=outp[h],
                                             lhsT=v2[0:n, kt, :],
                                             rhs=es[h][0:n, :],
                                             start=(ti == 0),
                                             stop=(ti == last_ti))
                    recs = {}
                    for h in pair:
                        rel_unit(c, h, outp[h], recs)
                    norm_pair(c, pair, recs)
                    for _ in range(2):
                        if fillers:
                            fillers.pop(0)()

                fillers += [(lambda tt=c * 4 + tt, nn=nn: outproj_unit(tt, nn))
                            for tt in range(4) for nn in range(4)]

            while fillers:
                fillers.pop(0)()
    nc.finalize()
    return nc


def _host_inputs(x, Wq_sem, Wk_sem, Wq_geo, Wk_geo, Wv, Wo, logit_scale):
    """Build the 8 per-core input maps."""
    import ml_dtypes
    bf16 = ml_dtypes.bfloat16
    pos = np.arange(T, dtype=np.float64)
    inv = 1.0 / (ROPE_BASE ** (np.arange(0, DG, 2, dtype=np.float64) / DG))
    ang = pos[:, None] * inv[None, :]              # [T, 16]
    cosT = np.cos(ang).T.astype(np.float32)        # [16, T]
    sinT = np.sin(ang).T.astype(np.float32)
    c32 = np.concatenate([cosT, cosT], axis=0)     # [32, T]
    s32 = np.concatenate([-sinT, sinT], axis=0)
    kpos = np.full(KPAD, 1e9, dtype=np.float32)
    kpos[:NBLK] = np.arange(NBLK) * MB + (MB - 1)
    kpos[128:] = np.arange(REMOTE, T)
    qpos = np.arange(T, dtype=np.float32)
    ident = np.eye(64, dtype=np.float32)
    xTs = [np.ascontiguousarray(x[b].T).astype(bf16) for b in range(B)]

    scale = np.exp(logit_scale.astype(np.float64)).astype(np.float32)
    in_maps = []
    for core in range(8):
        b, g = core // 4, core % 4
        W = np.empty((D, 384), dtype=np.float32)
        for h in range(4):
            gh = 4 * g + h
            s = scale[gh] / np.sqrt(np.float32(DS))
            W[:, h * 32:(h + 1) * 32] = Wq_sem[:, gh * DS:(gh + 1) * DS] * s
            W[:, 128 + 32 * h:128 + 32 * h + 16] = \
                Wq_geo[:, gh * DG:gh * DG + 16] * s
            W[:, 128 + 32 * h + 16:128 + 32 * (h + 1)] = \
                Wq_geo[:, gh * DG + 16:(gh + 1) * DG] * s
        W[:, 256:288] = Wk_sem[:, g * DS:(g + 1) * DS]
        W[:, 288:304] = Wk_geo[:, g * DG:g * DG + 16]
        W[:, 304:320] = Wk_geo[:, g * DG + 16:(g + 1) * DG]
        W[:, 320:384] = Wv[:, g * DV:(g + 1) * DV]
        in_maps.append({
            "xT": xTs[b],
            "W_all": W.astype(bf16),
            "Wo": np.ascontiguousarray(Wo[g * 256:(g + 1) * 256, :]).astype(bf16),
            "c32d": c32.astype(bf16), "s32d": s32.astype(bf16),
            "kpos": kpos, "qpos": qpos,
            "ident": ident,
        })
    return in_maps


def kernel(x, Wq_sem, Wk_sem, Wq_geo, Wk_geo, Wv, Wo, logit_scale, _trace=False):
    global _PROG
    import sys
    if "/opt/trn_rl_repo" not in sys.path:
        sys.path.insert(0, "/opt/trn_rl_repo")
    from concourse.bass_utils import run_bass_kernel_spmd

    x = np.asarray(x, dtype=np.float32)
    in_maps = _host_inputs(np.asarray(x, np.float32),
                           np.asarray(Wq_sem, np.float32),
                           np.asarray(Wk_sem, np.float32),
                           np.asarray(Wq_geo, np.float32),
                           np.asarray(Wk_geo, np.float32),
                           np.asarray(Wv, np.float32),
                           np.asarray(Wo, np.float32),
                           np.asarray(logit_scale, np.float32))
    if _PROG is None:
        _PROG = _build_program()
    res = run_bass_kernel_spmd(_PROG, in_maps, list(range(8)), trace=_trace)
    outs = [res.results[i]["y"].astype(np.float32) for i in range(8)]
    out = np.empty((B, T, D), dtype=np.float32)
    for b in range(B):
        out[b] = outs[4 * b] + outs[4 * b + 1] + outs[4 * b + 2] + outs[4 * b + 3]
    if _trace:
        return out, res
    return out


# revision 28
# speedup vs baseline: 1.0735x; 1.0735x over previous
"""Trainium2 Bass kernel for the decoupled sparse-attention layer.

Sharding: 8 cores = 2 batch x 4 GQA head-groups. Core i handles batch
b=i//4 and query heads [4g..4g+4) with KV head g, g=i%4. Each core
computes a partial output y_partial = attn_heads @ Wo_rows(group); the
host sums the 4 group partials per batch element.

Schedule (v3 — fused pipeline):
  Phase 1: projection GEMM stream for all 8 t-chunks + RoPE + incremental
    KV pooling, with the attention for query chunks 0..5 (which only see
    pooled mem-block keys) fused into the stream: attention for chunk c
    is issued as closures popped between the projection matmuls of chunk
    c+2, so its engine work (exp/mask/normalize) hides under the
    PE-dense projection stream. The pooled-V transpose re-transposes the
    whole prefix each chunk (same PE cost; keeps the v2 write at
    partition 0).
  Tail: attention for chunks 6,7 (the key-heavy ones) with all 8 chunks'
    out-projection matmuls interleaved as PE fillers. Softmax
    reciprocals via DVE reciprocal_approx_fast on an SBUF-staged
    denominator row; normalization is fused into the PSUM read with one
    scalar_tensor_tensor per head. PSUM->SBUF output casts alternate
    between ACT and DVE.

Per-core layouts (feature dim on partitions):
  xT      [2048, 4096] bf16 input activations (host pre-transposes)
  W_all   [2048, 384] bf16 fused projection weights, output cols:
            [0:128)   q_sem 4 heads x 32, scaled by exp(ls_h)/sqrt(32)
            [128:256) q_geo head-major [x1(16)|x2(16)] per head, scaled
            [256:288) k_sem 32
            [288:320) k_geo [x1(16)|x2(16)]
            [320:384) v 64
  q01_all/q23_all [128, T] bf16: per 64-row head slot [sem 32|x1' 16|x2' 16]
  Keys: 1152 padded slots = [48 mem-blocks | 80 pad | 1024 local].
"""

import numpy as np

B, T, D = 2, 4096, 2048
H, HKV, DS, DG, DV = 16, 4, 32, 32, 64
MB, LW = 64, 1024
REMOTE = T - LW            # 3072
NBLK = REMOTE // MB        # 48
NKEY = NBLK + LW           # 1072
KPAD = 128 + LW            # 1152 padded key slots
NKT = KPAD // 128          # 9 key tiles
TC = 512                   # t-chunk size
NC_CHUNKS = T // TC        # 8
ROPE_BASE = 10000.0

_PROG = None

SWAP16 = list(range(16, 32)) + list(range(0, 16))

# per-chunk count (of 16 outproj casts) assigned to the scalar engine
ACT_CAST_SHARE = [5, 5, 5, 5, 5, 5, 5, 8]


def _active_tiles(c):
    """Key tiles (tile_idx, nrows) visible to query chunk c, plus which
    tiles need the mask path."""
    tiles = [(0, 8 * (c + 1))] if c <= 5 else [(0, NBLK)]
    if c >= 6:
        nloc = (c - 5) * TC
        for t in range(1, 1 + nloc // 128):
            tiles.append((t, 128))
    masked = set()
    if c <= 5:
        masked.add(0)
    else:
        for t, n in tiles[1:]:
            maxpos = REMOTE + t * 128 - 1
            if maxpos > 512 * c:
                masked.add(t)
    return tiles, masked


def _build_program():
    from contextlib import ExitStack
    import concourse.bass as bass
    import concourse.bacc as bacc
    import concourse.tile as tile
    from concourse import mybir

    f32 = mybir.dt.float32
    bf16 = mybir.dt.bfloat16
    Alu = mybir.AluOpType
    Act = mybir.ActivationFunctionType

    nc = bacc.Bacc()
    xT = nc.declare_dram_parameter("xT", [D, T], bf16, isOutput=False)
    W_all = nc.declare_dram_parameter("W_all", [D, 384], bf16, isOutput=False)
    Wo = nc.declare_dram_parameter("Wo", [256, D], bf16, isOutput=False)
    c32d = nc.declare_dram_parameter("c32d", [32, T], bf16, isOutput=False)
    s32d = nc.declare_dram_parameter("s32d", [32, T], bf16, isOutput=False)
    kpos = nc.declare_dram_parameter("kpos", [KPAD], f32, isOutput=False)
    qpos = nc.declare_dram_parameter("qpos", [T], f32, isOutput=False)
    ident = nc.declare_dram_parameter("ident", [64, 64], f32, isOutput=False)
    y = nc.declare_dram_parameter("y", [T, D], bf16, isOutput=True)

    with tile.TileContext(nc) as tc, ExitStack() as ctx:
        persist = ctx.enter_context(tc.tile_pool(name="persist", bufs=1))
        xpool = ctx.enter_context(tc.tile_pool(name="x", bufs=3))
        tmp = ctx.enter_context(tc.tile_pool(name="tmp", bufs=2))
        epool = ctx.enter_context(tc.tile_pool(name="e", bufs=4))
        mpool = ctx.enter_context(tc.tile_pool(name="m", bufs=4))
        ypool = ctx.enter_context(tc.tile_pool(name="y", bufs=3))
        npool = ctx.enter_context(tc.tile_pool(name="n", bufs=2))

        # ---- persistent SBUF tensors ----
        wall_sb = persist.tile([128, 16, 384], bf16)

        def wall_dma(wh):
            nc.sync.dma_start(
                out=wall_sb[:, 4 * wh:4 * wh + 4, :],
                in_=bass.AP(tensor=W_all, offset=wh * 4 * 128 * 384,
                            ap=[[384, 128], [384 * 128, 4], [1, 384]]))

        wo_sb = persist.tile([128, 2, D], bf16)
        C128 = persist.tile([128, T], bf16)
        S128 = persist.tile([128, T], bf16)
        ident_sb = persist.tile([64, 64], f32)
        nc.scalar.dma_start(out=ident_sb, in_=ident[:, :])
        kpos_sb = persist.tile([128, NKT], f32)
        nc.scalar.dma_start(
            out=kpos_sb,
            in_=bass.AP(tensor=kpos, offset=0, ap=[[1, 128], [128, NKT]]))
        qpos_all = persist.tile([128, T], f32)

        q01_all = persist.tile([128, T], bf16)
        q23_all = persist.tile([128, T], bf16)
        aT01 = persist.tile([128, T], bf16)
        aT23 = persist.tile([128, T], bf16)
        kTd = persist.tile([128, KPAD], bf16)   # [sem32|x1'16|x2'16] dup'd
        vT = persist.tile([64, KPAD], f32)
        nc.vector.memset(vT, 0.0)
        v2 = persist.tile([128, NKT, 65], bf16)  # [key, dv | ones]
        onesrc = persist.tile([128, 1], f32)
        nc.vector.memset(onesrc, 1.0)
        nc.vector.tensor_copy(out=v2[0:NBLK, 0, 64:65], in_=onesrc[0:NBLK, :])
        for t in range(1, NKT):
            nc.vector.tensor_copy(out=v2[:, t, 64:65], in_=onesrc)
        vsum = persist.tile([64, 1], f32)

        qsrc = [q01_all, q01_all, q23_all, q23_all]
        qb4 = [0, 64, 0, 64]          # 64-row slot base per head
        aTs = [aT01, aT01, aT23, aT23]

        def rel_unit(c, h, outp_h, recs):
            """Free an outp bank fast: den stage + unnormalized aT cast on
            ACT, reciprocal on DVE."""
            sl = slice(c * TC, (c + 1) * TC)
            hh = h % 2
            den = npool.tile([1, TC], f32, tag=f"den{hh}", bufs=1,
                             name=f"den_{c}_{h}")
            nc.scalar.copy(out=den, in_=outp_h[64:65, :])
            base = qb4[h]
            nc.scalar.copy(out=aTs[h][base:base + 64, sl],
                           in_=outp_h[0:64, :])
            rec = npool.tile([1, TC], f32, tag=f"rec{hh}", bufs=1,
                             name=f"rec_{c}_{h}")
            nc.vector.reciprocal_approx_fast(out=rec, in_=den)
            recs[h] = rec

        def norm_pair(c, pair, recs):
            """Trailing normalize: assemble the pair's [128,TC] reciprocal
            broadcast (both TT inputs must share base partition 0) and one
            in-place multiply."""
            sl = slice(c * TC, (c + 1) * TC)
            rbP = npool.tile([128, TC], f32, tag="rbP", bufs=2,
                             name=f"rbP_{c}_{pair[0]}")
            rbt = npool.tile([64, TC], f32, tag="rbt", bufs=2,
                             name=f"rbt_{c}_{pair[0]}")
            nc.gpsimd.partition_broadcast(out_ap=rbP[0:64, :],
                                          in_ap=recs[pair[0]])
            nc.gpsimd.partition_broadcast(out_ap=rbt, in_ap=recs[pair[1]])
            nc.vector.tensor_copy(out=rbP[64:128, :], in_=rbt)
            dst = aT01 if pair[0] < 2 else aT23
            nc.vector.tensor_mul(dst[:, sl], dst[:, sl], rbP)

        # ========== PHASE 1: projections + fused attention (chunks 0-5) ====
        with tc.tile_pool(name="psp", bufs=2, space="PSUM") as ps_proj, \
             tc.tile_pool(name="psatt", bufs=1, space="PSUM") as ps_att:

            def fused_attn_closures(cq):
                """Attention for query chunk cq (pooled keys only), issued
                as closures popped between chunk cq+2's projection matmuls."""
                lo = cq * TC
                sl = slice(lo, lo + TC)
                n = 8 * (cq + 1)
                m_sb = mpool.tile([128, TC], bf16, tag="mask", bufs=6,
                                  name=f"mf{cq}")
                nc.vector.tensor_scalar(
                    out=m_sb[0:n, :], in0=qpos_all[0:n, sl],
                    scalar1=kpos_sb[0:n, 0:1], scalar2=None, op0=Alu.is_ge)
                es = {}
                outp = {}

                def vtr():
                    pv = ps_att.tile([128, 64], f32, tag="sc", name=f"pvf{cq}")
                    nc.tensor.transpose(out=pv[0:n, :], in_=vT[:, 0:n],
                                        identity=ident_sb)
                    nc.vector.tensor_copy(out=v2[0:n, 0, 0:64], in_=pv[0:n, :])

                def mk_sc(h):
                    def f():
                        qb = qb4[h]
                        sc = ps_att.tile([128, TC], f32, tag="sc",
                                         name=f"scf{cq}_{h}")
                        nc.tensor.matmul(out=sc[0:n, :],
                                         lhsT=kTd[qb:qb + 64, 0:n],
                                         rhs=qsrc[h][qb:qb + 64, sl],
                                         start=True, stop=True)
                        e_sb = epool.tile([128, TC], bf16, tag="e",
                                          name=f"ef{cq}_{h}")
                        nc.scalar.activation(out=e_sb[0:n, :], in_=sc[0:n, :],
                                             func=Act.Exp)
                        nc.vector.tensor_mul(e_sb[0:n, :], e_sb[0:n, :],
                                             m_sb[0:n, :])
                        es[h] = e_sb
                    return f

                def mk_pv(h):
                    def f():
                        outp[h] = ps_att.tile([65, TC], f32, tag="outp",
                                              name=f"outpf{cq}_{h}")
                        nc.tensor.matmul(out=outp[h], lhsT=v2[0:n, 0, :],
                                         rhs=es[h][0:n, :],
                                         start=True, stop=True)
                    return f

                recs = {}

                def mk_fin(h):
                    return lambda: rel_unit(cq, h, outp[h], recs)

                def mk_norm(pair):
                    return lambda: norm_pair(cq, pair, recs)

                return [vtr, mk_sc(0), mk_pv(0), mk_sc(1), mk_fin(0),
                        mk_pv(1), mk_sc(2), mk_fin(1), mk_norm((0, 1)),
                        mk_pv(2), mk_sc(3), mk_fin(2), mk_pv(3), mk_fin(3),
                        mk_norm((2, 3))]

            pend, nextp = [], []
            xts = {}
            for c in range(NC_CHUNKS):
                lo = c * TC
                sl = slice(lo, lo + TC)
                xt = xts[c] if c == 1 else xpool.tile([128, 16, TC], bf16,
                                                       tag="xt")
                if c == 0:
                    # interleave x and W pieces so the first matmul's inputs
                    # land first (startup is HBM-bandwidth-bound)
                    def xt_dma(kq):
                        nc.sync.dma_start(
                            out=xt[:, 4 * kq:4 * kq + 4, :],
                            in_=bass.AP(tensor=xT, offset=kq * 4 * 128 * T + lo,
                                        ap=[[T, 128], [T * 128, 4], [1, TC]]))
                    xt_dma(0); wall_dma(0); wall_dma(1); xt_dma(1)
                    wall_dma(2); wall_dma(3); xt_dma(2); xt_dma(3)
                    # prefetch chunk 1's x so the c0->c1 boundary has DMA lead
                    xts[1] = xpool.tile([128, 16, TC], bf16, tag="xt",
                                        name="xt1pf")
                    for xh in range(2):
                        nc.sync.dma_start(
                            out=xts[1][:, 8 * xh:8 * xh + 8, :],
                            in_=bass.AP(tensor=xT,
                                        offset=xh * 8 * 128 * T + TC,
                                        ap=[[T, 128], [T * 128, 8], [1, TC]]))
                    # cos/sin: one HBM load + on-chip replication (SB->SB DMA)
                    nc.scalar.dma_start(out=C128[0:32, :], in_=c32d[:, :])
                    nc.scalar.dma_start(out=S128[0:32, :], in_=s32d[:, :])
                    for qd in range(1, 4):
                        nc.scalar.dma_start(out=C128[32 * qd:32 * qd + 32, :],
                                            in_=C128[0:32, :])
                        nc.scalar.dma_start(out=S128[32 * qd:32 * qd + 32, :],
                                            in_=S128[0:32, :])
                elif c == 1:
                    nc.scalar.dma_start(
                        out=wo_sb,
                        in_=bass.AP(tensor=Wo, offset=0,
                                    ap=[[D, 128], [D * 128, 2], [1, D]]))
                else:
                    for xh in range(2):
                        nc.sync.dma_start(
                            out=xt[:, 8 * xh:8 * xh + 8, :],
                            in_=bass.AP(tensor=xT,
                                        offset=xh * 8 * 128 * T + lo,
                                        ap=[[T, 128], [T * 128, 8], [1, TC]]))
                if c <= 3:
                    # qpos pieces 2c, 2c+1 (needed by fused masks from chunk c)
                    for qq in range(2):
                        qlo = (2 * c + qq) * TC
                        nc.scalar.dma_start(
                            out=qpos_all[:, qlo:qlo + TC],
                            in_=bass.AP(tensor=qpos, offset=qlo,
                                        ap=[[0, 128], [1, TC]]))
                psA = ps_proj.tile([128, TC], f32, tag="psA")
                psB = ps_proj.tile([128, TC], f32, tag="psB")
                psC = ps_proj.tile([128, TC], f32, tag="psC")
                for kk in range(16):
                    st, sp = kk == 0, kk == 15
                    w = wall_sb[:, kk, :]
                    xk = xt[:, kk, :]
                    nc.tensor.matmul(out=psA, lhsT=w[:, 0:128], rhs=xk,
                                     start=st, stop=sp)
                    nc.tensor.matmul(out=psB, lhsT=w[:, 128:256], rhs=xk,
                                     start=st, stop=sp)
                    nc.tensor.matmul(out=psC, lhsT=w[:, 256:384], rhs=xk,
                                     start=st, stop=sp)
                    if pend:
                        pend.pop(0)()

                # q_sem copies into the 64-row head slots (ACT, bf16 out)
                nc.scalar.copy(out=q01_all[0:32, sl], in_=psA[0:32, :])
                nc.scalar.copy(out=q01_all[64:96, sl], in_=psA[32:64, :])
                nc.scalar.copy(out=q23_all[0:32, sl], in_=psA[64:96, :])
                nc.scalar.copy(out=q23_all[64:96, sl], in_=psA[96:128, :])
                # q_geo RoPE for all 4 heads on DVE
                swq = tmp.tile([128, TC], f32, tag="swq")
                t1q = tmp.tile([128, TC], bf16, tag="t1q")
                sw2 = tmp.tile([128, TC], bf16, tag="sw2")
                nc.vector.stream_shuffle(out=swq, in_=psB, mask=SWAP16)
                nc.vector.tensor_mul(t1q, psB, C128[:, sl])
                nc.vector.tensor_mul(sw2, swq, S128[:, sl])
                for h in range(4):
                    nc.vector.tensor_add(
                        qsrc[h][qb4[h] + 32:qb4[h] + 64, sl],
                        t1q[32 * h:32 * h + 32, :], sw2[32 * h:32 * h + 32, :])

                # k side: rope geo, then pool (c<=5) or copy local (c>=6)
                swp = tmp.tile([64, TC], f32, tag="swp")
                t1 = tmp.tile([32, TC], bf16, tag="t1")
                t2 = tmp.tile([32, TC], bf16, tag="t2")
                blk = psC[32:64, :]
                nc.vector.stream_shuffle(out=swp[32:64, :], in_=blk, mask=SWAP16)
                nc.vector.tensor_mul(t1, blk, C128[0:32, sl])
                nc.vector.tensor_mul(t2, swp[32:64, :], S128[32:64, sl])
                if c <= 5:
                    ktmp = tmp.tile([64, TC], bf16, tag="ktmp")
                    nc.scalar.copy(out=ktmp[0:32, :], in_=psC[0:32, :])
                    nc.vector.tensor_add(ktmp[32:64, :], t1, t2)
                    bs = slice(c * 8, (c + 1) * 8)
                    with nc.allow_low_precision(reason="bf16 pooled keys"):
                        nc.vector.tensor_reduce(
                            out=kTd[0:64, bs],
                            in_=ktmp.rearrange("p (n w) -> p n w", w=MB),
                            axis=mybir.AxisListType.X, op=Alu.add)
                    nc.vector.tensor_scalar_mul(kTd[0:64, bs], kTd[0:64, bs], 1.0 / MB)
                    nc.vector.tensor_reduce(
                        out=vT[:, bs],
                        in_=psC[64:128, :].rearrange("p (n w) -> p n w", w=MB),
                        axis=mybir.AxisListType.X, op=Alu.add)
                    nc.vector.tensor_scalar_mul(vT[:, bs], vT[:, bs], 1.0 / MB)
                    # incremental kTd dup for the new pooled cols
                    nc.scalar.copy(out=kTd[64:128, bs], in_=kTd[0:64, bs])
                else:
                    loff = 128 + (c - 6) * TC
                    lsl = slice(loff, loff + TC)
                    nc.scalar.copy(out=kTd[0:32, lsl], in_=psC[0:32, :])
                    nc.vector.tensor_add(kTd[32:64, lsl], t1, t2)
                    nc.scalar.copy(out=vT[:, lsl], in_=psC[64:128, :])

                while pend:
                    pend.pop(0)()
                pend = nextp
                nextp = fused_attn_closures(c) if c <= 5 else []
            while pend:
                pend.pop(0)()

        # ===== TAIL: attention chunks 6,7 + all out-projections =====
        with tc.tile_pool(name="pssc", bufs=2, space="PSUM") as ps_sc, \
             tc.tile_pool(name="psout", bufs=2, space="PSUM") as ps_out, \
             tc.tile_pool(name="psy", bufs=4, space="PSUM") as ps_y:

            # local kTd dup (cols 128..1152) — needed by chunks 6,7
            nc.scalar.copy(out=kTd[64:128, 128:KPAD], in_=kTd[0:64, 128:KPAD])
            # vsum + uniform-row patch for chunk 0 (queries with no visible key)
            nc.vector.tensor_reduce(out=vsum, in_=vT, axis=mybir.AxisListType.X,
                                    op=Alu.add)
            nc.vector.tensor_scalar_mul(vsum, vsum, 1.0 / float(NKEY))
            for dst in (aT01, aT23):
                for base in (0, 64):
                    nc.vector.tensor_copy(
                        out=dst[base:base + 64, 0:63],
                        in_=vsum.broadcast_to([64, 63]))

            # local V transposes into v2 — issued as early fillers
            def v2_unit(i):
                pv = ps_sc.tile([128, 64], f32, tag="sc", name=f"pv{i}")
                nc.tensor.transpose(out=pv,
                                    in_=vT[:, 128 + 128 * i:256 + 128 * i],
                                    identity=ident_sb)
                nc.vector.tensor_copy(out=v2[:, 1 + i, 0:64], in_=pv)

            ysb = {}
            cast_ctr = [0, 0]   # [chunk-local cast idx, chunk]

            def outproj_unit(tt, nn):
                tsl = slice(tt * 128, (tt + 1) * 128)
                nsl = slice(nn * 512, (nn + 1) * 512)
                yp = ps_y.tile([128, 512], f32, tag="yp")
                nc.tensor.matmul(out=yp, lhsT=aT01[:, tsl],
                                 rhs=wo_sb[:, 0, nsl], start=True, stop=False)
                nc.tensor.matmul(out=yp, lhsT=aT23[:, tsl],
                                 rhs=wo_sb[:, 1, nsl], start=False, stop=True)
                if tt not in ysb:
                    ysb[tt] = ypool.tile([128, D], bf16, tag="ysb",
                                         name=f"ysb{tt}")
                cc = tt // 4
                if cast_ctr[1] != cc:
                    cast_ctr[0], cast_ctr[1] = 0, cc
                on_act = cast_ctr[0] < ACT_CAST_SHARE[cc]
                cast_ctr[0] += 1
                if on_act:
                    nc.scalar.copy(out=ysb[tt][:, nsl], in_=yp)
                else:
                    nc.vector.tensor_copy(out=ysb[tt][:, nsl], in_=yp)
                if nn == 3:
                    nc.sync.dma_start(out=y[tsl, :], in_=ysb.pop(tt))

            fillers = [(lambda i=i: v2_unit(i)) for i in range(8)]
            fillers += [(lambda tt=tt, nn=nn: outproj_unit(tt, nn))
                        for tt in range(24) for nn in range(4)]
            for c in (6, 7):
                lo = c * TC
                sl = slice(lo, lo + TC)
                tiles, masked = _active_tiles(c)
                mdict = {}
                for (mt, n) in tiles:
                    if mt in masked:
                        m_sb = mpool.tile([128, TC], bf16, tag="mask", bufs=6)
                        nc.vector.tensor_scalar(
                            out=m_sb[0:n, :], in0=qpos_all[0:n, sl],
                            scalar1=kpos_sb[0:n, mt:mt + 1], scalar2=None,
                            op0=Alu.is_ge)
                        mdict[mt] = m_sb

                outp = {}
                last_ti = len(tiles) - 1

                for pair in ((0, 1), (2, 3)):
                    for h in pair:
                        outp[h] = ps_out.tile([65, TC], f32, tag="out",
                                              name=f"outp{c}_{h}")
                    for ti, (kt, n) in enumerate(tiles):
                        ks = slice(kt * 128, kt * 128 + n)
                        es = {}
                        for h in pair:
                            qb = qb4[h]
                            sc = ps_sc.tile([128, TC], f32, tag="sc",
                                            name=f"sc{c}_{h}_{kt}")
                            nc.tensor.matmul(out=sc[0:n, :],
                                             lhsT=kTd[qb:qb + 64, ks],
                                             rhs=qsrc[h][qb:qb + 64, sl],
                                             start=True, stop=True)
                            e_sb = epool.tile([128, TC], bf16, tag="e",
                                              name=f"e{c}_{h}_{kt}")
                            nc.scalar.activation(out=e_sb[0:n, :],
                                                 in_=sc[0:n, :], func=Act.Exp)
                            if kt in mdict:
                                nc.vector.tensor_mul(
                                    e_sb[0:n, :], e_sb[0:n, :],
                                    mdict[kt][0:n, :])
                            es[h] = e_sb
                        for _ in range(4):
                            if fillers:
                                fillers.pop(0)()
                        for h in pair:
                            nc.tensor.matmul(out=outp[h],
                                             lhsT=v2[0:n, kt, :],
                                             rhs=es[h][0:n, :],
                                             start=(ti == 0),
                                             stop=(ti == last_ti))
                    recs = {}
                    for h in pair:
                        rel_unit(c, h, outp[h], recs)
                    norm_pair(c, pair, recs)

                fillers += [(lambda tt=c * 4 + tt, nn=nn: outproj_unit(tt, nn))
                            for tt in range(4) for nn in range(4)]

            while fillers:
                fillers.pop(0)()
    nc.finalize()
    return nc


def _host_inputs(x, Wq_sem, Wk_sem, Wq_geo, Wk_geo, Wv, Wo, logit_scale):
    """Build the 8 per-core input maps."""
    import ml_dtypes
    bf16 = ml_dtypes.bfloat16
    pos = np.arange(T, dtype=np.float64)
    inv = 1.0 / (ROPE_BASE ** (np.arange(0, DG, 2, dtype=np.float64) / DG))
    ang = pos[:, None] * inv[None, :]              # [T, 16]
    cosT = np.cos(ang).T.astype(np.float32)        # [16, T]
    sinT = np.sin(ang).T.astype(np.float32)
    c32 = np.concatenate([cosT, cosT], axis=0)     # [32, T]
    s32 = np.concatenate([-sinT, sinT], axis=0)
    kpos = np.full(KPAD, 1e9, dtype=np.float32)
    kpos[:NBLK] = np.arange(NBLK) * MB + (MB - 1)
    kpos[128:] = np.arange(REMOTE, T)
    qpos = np.arange(T, dtype=np.float32)
    ident = np.eye(64, dtype=np.float32)
    xTs = [np.ascontiguousarray(x[b].T).astype(bf16) for b in range(B)]

    scale = np.exp(logit_scale.astype(np.float64)).astype(np.float32)
    in_maps = []
    for core in range(8):
        b, g = core // 4, core % 4
        W = np.empty((D, 384), dtype=np.float32)
        for h in range(4):
            gh = 4 * g + h
            s = scale[gh] / np.sqrt(np.float32(DS))
            W[:, h * 32:(h + 1) * 32] = Wq_sem[:, gh * DS:(gh + 1) * DS] * s
            W[:, 128 + 32 * h:128 + 32 * h + 16] = \
                Wq_geo[:, gh * DG:gh * DG + 16] * s
            W[:, 128 + 32 * h + 16:128 + 32 * (h + 1)] = \
                Wq_geo[:, gh * DG + 16:(gh + 1) * DG] * s
        W[:, 256:288] = Wk_sem[:, g * DS:(g + 1) * DS]
        W[:, 288:304] = Wk_geo[:, g * DG:g * DG + 16]
        W[:, 304:320] = Wk_geo[:, g * DG + 16:(g + 1) * DG]
        W[:, 320:384] = Wv[:, g * DV:(g + 1) * DV]
        in_maps.append({
            "xT": xTs[b],
            "W_all": W.astype(bf16),
            "Wo": np.ascontiguousarray(Wo[g * 256:(g + 1) * 256, :]).astype(bf16),
            "c32d": c32.astype(bf16), "s32d": s32.astype(bf16),
            "kpos": kpos, "qpos": qpos,
            "ident": ident,
        })
    return in_maps


def kernel(x, Wq_sem, Wk_sem, Wq_geo, Wk_geo, Wv, Wo, logit_scale, _trace=False):
    global _PROG
    import sys
    if "/opt/trn_rl_repo" not in sys.path:
        sys.path.insert(0, "/opt/trn_rl_repo")
    from concourse.bass_utils import run_bass_kernel_spmd

    x = np.asarray(x, dtype=np.float32)
    in_maps = _host_inputs(np.asarray(x, np.float32),
                           np.asarray(Wq_sem, np.float32),
                           np.asarray(Wk_sem, np.float32),
                           np.asarray(Wq_geo, np.float32),
                           np.asarray(Wk_geo, np.float32),
                           np.asarray(Wv, np.float32),
                           np.asarray(Wo, np.float32),
                           np.asarray(logit_scale, np.float32))
    if _PROG is None:
        _PROG = _build_program()
    res = run_bass_kernel_spmd(_PROG, in_maps, list(range(8)), trace=_trace)
    outs = [res.results[i]["y"].astype(np.float32) for i in range(8)]
    out = np.empty((B, T, D), dtype=np.float32)
    for b in range(B):
        out[b] = outs[4 * b] + outs[4 * b + 1] + outs[4 * b + 2] + outs[4 * b + 3]
    if _trace:
        return out, res
    return out


# revision 29
# speedup vs baseline: 1.0737x; 1.0003x over previous
"""Trainium2 Bass kernel for the decoupled sparse-attention layer.

Sharding: 8 cores = 2 batch x 4 GQA head-groups. Core i handles batch
b=i//4 and query heads [4g..4g+4) with KV head g, g=i%4. Each core
computes a partial output y_partial = attn_heads @ Wo_rows(group); the
host sums the 4 group partials per batch element.

Schedule (v3 — fused pipeline):
  Phase 1: projection GEMM stream for all 8 t-chunks + RoPE + incremental
    KV pooling, with the attention for query chunks 0..5 (which only see
    pooled mem-block keys) fused into the stream: attention for chunk c
    is issued as closures popped between the projection matmuls of chunk
    c+2, so its engine work (exp/mask/normalize) hides under the
    PE-dense projection stream. The pooled-V transpose re-transposes the
    whole prefix each chunk (same PE cost; keeps the v2 write at
    partition 0).
  Tail: attention for chunks 6,7 (the key-heavy ones) with all 8 chunks'
    out-projection matmuls interleaved as PE fillers. Softmax
    reciprocals via DVE reciprocal_approx_fast on an SBUF-staged
    denominator row; normalization is fused into the PSUM read with one
    scalar_tensor_tensor per head. PSUM->SBUF output casts alternate
    between ACT and DVE.

Per-core layouts (feature dim on partitions):
  xT      [2048, 4096] bf16 input activations (host pre-transposes)
  W_all   [2048, 384] bf16 fused projection weights, output cols:
            [0:128)   q_sem 4 heads x 32, scaled by exp(ls_h)/sqrt(32)
            [128:256) q_geo head-major [x1(16)|x2(16)] per head, scaled
            [256:288) k_sem 32
            [288:320) k_geo [x1(16)|x2(16)]
            [320:384) v 64
  q01_all/q23_all [128, T] bf16: per 64-row head slot [sem 32|x1' 16|x2' 16]
  Keys: 1152 padded slots = [48 mem-blocks | 80 pad | 1024 local].
"""

import numpy as np

B, T, D = 2, 4096, 2048
H, HKV, DS, DG, DV = 16, 4, 32, 32, 64
MB, LW = 64, 1024
REMOTE = T - LW            # 3072
NBLK = REMOTE // MB        # 48
NKEY = NBLK + LW           # 1072
KPAD = 128 + LW            # 1152 padded key slots
NKT = KPAD // 128          # 9 key tiles
TC = 512                   # t-chunk size
NC_CHUNKS = T // TC        # 8
ROPE_BASE = 10000.0

_PROG = None

SWAP16 = list(range(16, 32)) + list(range(0, 16))

# per-chunk count (of 16 outproj casts) assigned to the scalar engine
ACT_CAST_SHARE = [5, 5, 5, 5, 5, 5, 5, 8]


def _active_tiles(c):
    """Key tiles (tile_idx, nrows) visible to query chunk c, plus which
    tiles need the mask path."""
    tiles = [(0, 8 * (c + 1))] if c <= 5 else [(0, NBLK)]
    if c >= 6:
        nloc = (c - 5) * TC
        for t in range(1, 1 + nloc // 128):
            tiles.append((t, 128))
    masked = set()
    if c <= 5:
        masked.add(0)
    else:
        for t, n in tiles[1:]:
            maxpos = REMOTE + t * 128 - 1
            if maxpos > 512 * c:
                masked.add(t)
    return tiles, masked


def _build_program():
    from contextlib import ExitStack
    import concourse.bass as bass
    import concourse.bacc as bacc
    import concourse.tile as tile
    from concourse import mybir

    f32 = mybir.dt.float32
    bf16 = mybir.dt.bfloat16
    Alu = mybir.AluOpType
    Act = mybir.ActivationFunctionType

    nc = bacc.Bacc()
    xT = nc.declare_dram_parameter("xT", [D, T], bf16, isOutput=False)
    W_all = nc.declare_dram_parameter("W_all", [D, 384], bf16, isOutput=False)
    Wo = nc.declare_dram_parameter("Wo", [256, D], bf16, isOutput=False)
    c32d = nc.declare_dram_parameter("c32d", [32, T], bf16, isOutput=False)
    s32d = nc.declare_dram_parameter("s32d", [32, T], bf16, isOutput=False)
    kpos = nc.declare_dram_parameter("kpos", [KPAD], f32, isOutput=False)
    qpos = nc.declare_dram_parameter("qpos", [T], f32, isOutput=False)
    ident = nc.declare_dram_parameter("ident", [64, 64], f32, isOutput=False)
    y = nc.declare_dram_parameter("y", [T, D], bf16, isOutput=True)

    with tile.TileContext(nc) as tc, ExitStack() as ctx:
        persist = ctx.enter_context(tc.tile_pool(name="persist", bufs=1))
        xpool = ctx.enter_context(tc.tile_pool(name="x", bufs=3))
        tmp = ctx.enter_context(tc.tile_pool(name="tmp", bufs=2))
        epool = ctx.enter_context(tc.tile_pool(name="e", bufs=4))
        mpool = ctx.enter_context(tc.tile_pool(name="m", bufs=4))
        ypool = ctx.enter_context(tc.tile_pool(name="y", bufs=3))
        npool = ctx.enter_context(tc.tile_pool(name="n", bufs=2))

        # ---- persistent SBUF tensors ----
        wall_sb = persist.tile([128, 16, 384], bf16)

        def wall_dma(wh):
            nc.sync.dma_start(
                out=wall_sb[:, 4 * wh:4 * wh + 4, :],
                in_=bass.AP(tensor=W_all, offset=wh * 4 * 128 * 384,
                            ap=[[384, 128], [384 * 128, 4], [1, 384]]))

        wo_sb = persist.tile([128, 2, D], bf16)
        C128 = persist.tile([128, T], bf16)
        S128 = persist.tile([128, T], bf16)
        ident_sb = persist.tile([64, 64], f32)
        nc.scalar.dma_start(out=ident_sb, in_=ident[:, :])
        kpos_sb = persist.tile([128, NKT], f32)
        nc.scalar.dma_start(
            out=kpos_sb,
            in_=bass.AP(tensor=kpos, offset=0, ap=[[1, 128], [128, NKT]]))
        qpos_all = persist.tile([128, T], f32)

        q01_all = persist.tile([128, T], bf16)
        q23_all = persist.tile([128, T], bf16)
        aT01 = persist.tile([128, T], bf16)
        aT23 = persist.tile([128, T], bf16)
        kTd = persist.tile([128, KPAD], bf16)   # [sem32|x1'16|x2'16] dup'd
        vT = persist.tile([64, KPAD], f32)
        nc.vector.memset(vT, 0.0)
        v2 = persist.tile([128, NKT, 65], bf16)  # [key, dv | ones]
        onesrc = persist.tile([128, 1], f32)
        nc.vector.memset(onesrc, 1.0)
        nc.vector.tensor_copy(out=v2[0:NBLK, 0, 64:65], in_=onesrc[0:NBLK, :])
        for t in range(1, NKT):
            nc.vector.tensor_copy(out=v2[:, t, 64:65], in_=onesrc)
        vsum = persist.tile([64, 1], f32)

        qsrc = [q01_all, q01_all, q23_all, q23_all]
        qb4 = [0, 64, 0, 64]          # 64-row slot base per head
        aTs = [aT01, aT01, aT23, aT23]

        def rel_unit(c, h, outp_h, recs):
            """Free an outp bank fast: den stage + unnormalized aT cast on
            ACT, reciprocal on DVE."""
            sl = slice(c * TC, (c + 1) * TC)
            hh = h % 2
            den = npool.tile([1, TC], f32, tag=f"den{hh}", bufs=1,
                             name=f"den_{c}_{h}")
            nc.scalar.copy(out=den, in_=outp_h[64:65, :])
            base = qb4[h]
            nc.scalar.copy(out=aTs[h][base:base + 64, sl],
                           in_=outp_h[0:64, :])
            rec = npool.tile([1, TC], f32, tag=f"rec{hh}", bufs=1,
                             name=f"rec_{c}_{h}")
            nc.vector.reciprocal_approx_fast(out=rec, in_=den)
            recs[h] = rec

        def norm_pair(c, pair, recs):
            """Trailing normalize: assemble the pair's [128,TC] reciprocal
            broadcast (both TT inputs must share base partition 0) and one
            in-place multiply."""
            sl = slice(c * TC, (c + 1) * TC)
            rbP = npool.tile([128, TC], f32, tag="rbP", bufs=2,
                             name=f"rbP_{c}_{pair[0]}")
            rbt = npool.tile([64, TC], f32, tag="rbt", bufs=2,
                             name=f"rbt_{c}_{pair[0]}")
            nc.gpsimd.partition_broadcast(out_ap=rbP[0:64, :],
                                          in_ap=recs[pair[0]])
            nc.gpsimd.partition_broadcast(out_ap=rbt, in_ap=recs[pair[1]])
            nc.vector.tensor_copy(out=rbP[64:128, :], in_=rbt)
            dst = aT01 if pair[0] < 2 else aT23
            nc.vector.tensor_mul(dst[:, sl], dst[:, sl], rbP)

        # ========== PHASE 1: projections + fused attention (chunks 0-5) ====
        with tc.tile_pool(name="psp", bufs=2, space="PSUM") as ps_proj, \
             tc.tile_pool(name="psatt", bufs=1, space="PSUM") as ps_att:

            def fused_attn_closures(cq):
                """Attention for query chunk cq (pooled keys only), issued
                as closures popped between chunk cq+2's projection matmuls."""
                lo = cq * TC
                sl = slice(lo, lo + TC)
                n = 8 * (cq + 1)
                m_sb = mpool.tile([128, TC], bf16, tag="mask", bufs=6,
                                  name=f"mf{cq}")
                nc.vector.tensor_scalar(
                    out=m_sb[0:n, :], in0=qpos_all[0:n, sl],
                    scalar1=kpos_sb[0:n, 0:1], scalar2=None, op0=Alu.is_ge)
                es = {}
                outp = {}

                def vtr():
                    pv = ps_att.tile([128, 64], f32, tag="sc", name=f"pvf{cq}")
                    nc.tensor.transpose(out=pv[0:n, :], in_=vT[:, 0:n],
                                        identity=ident_sb)
                    nc.vector.tensor_copy(out=v2[0:n, 0, 0:64], in_=pv[0:n, :])

                def mk_sc(h):
                    def f():
                        qb = qb4[h]
                        sc = ps_att.tile([128, TC], f32, tag="sc",
                                         name=f"scf{cq}_{h}")
                        nc.tensor.matmul(out=sc[0:n, :],
                                         lhsT=kTd[qb:qb + 64, 0:n],
                                         rhs=qsrc[h][qb:qb + 64, sl],
                                         start=True, stop=True)
                        e_sb = epool.tile([128, TC], bf16, tag="e",
                                          name=f"ef{cq}_{h}")
                        nc.scalar.activation(out=e_sb[0:n, :], in_=sc[0:n, :],
                                             func=Act.Exp)
                        nc.vector.tensor_mul(e_sb[0:n, :], e_sb[0:n, :],
                                             m_sb[0:n, :])
                        es[h] = e_sb
                    return f

                def mk_pv(h):
                    def f():
                        outp[h] = ps_att.tile([65, TC], f32, tag="outp",
                                              name=f"outpf{cq}_{h}")
                        nc.tensor.matmul(out=outp[h], lhsT=v2[0:n, 0, :],
                                         rhs=es[h][0:n, :],
                                         start=True, stop=True)
                    return f

                recs = {}

                def mk_fin(h):
                    return lambda: rel_unit(cq, h, outp[h], recs)

                def mk_norm(pair):
                    return lambda: norm_pair(cq, pair, recs)

                return [vtr, mk_sc(0), mk_pv(0), mk_sc(1), mk_fin(0),
                        mk_pv(1), mk_sc(2), mk_fin(1), mk_norm((0, 1)),
                        mk_pv(2), mk_sc(3), mk_fin(2), mk_pv(3), mk_fin(3),
                        mk_norm((2, 3))]

            pend, nextp = [], []
            xts = {}
            for c in range(NC_CHUNKS):
                lo = c * TC
                sl = slice(lo, lo + TC)
                xt = xts[c] if c == 1 else xpool.tile([128, 16, TC], bf16,
                                                       tag="xt")
                if c == 0:
                    # interleave x and W pieces so the first matmul's inputs
                    # land first (startup is HBM-bandwidth-bound)
                    def xt_dma(kq):
                        nc.sync.dma_start(
                            out=xt[:, 4 * kq:4 * kq + 4, :],
                            in_=bass.AP(tensor=xT, offset=kq * 4 * 128 * T + lo,
                                        ap=[[T, 128], [T * 128, 4], [1, TC]]))
                    xt_dma(0); wall_dma(0); wall_dma(1); xt_dma(1)
                    wall_dma(2); wall_dma(3); xt_dma(2); xt_dma(3)
                    # prefetch chunk 1's x so the c0->c1 boundary has DMA lead
                    xts[1] = xpool.tile([128, 16, TC], bf16, tag="xt",
                                        name="xt1pf")
                    for xh in range(2):
                        nc.sync.dma_start(
                            out=xts[1][:, 8 * xh:8 * xh + 8, :],
                            in_=bass.AP(tensor=xT,
                                        offset=xh * 8 * 128 * T + TC,
                                        ap=[[T, 128], [T * 128, 8], [1, TC]]))
                    # cos/sin: one HBM load + on-chip replication (SB->SB DMA)
                    nc.scalar.dma_start(out=C128[0:32, :], in_=c32d[:, :])
                    nc.scalar.dma_start(out=S128[0:32, :], in_=s32d[:, :])
                    for qd in range(1, 4):
                        nc.scalar.dma_start(out=C128[32 * qd:32 * qd + 32, :],
                                            in_=C128[0:32, :])
                        nc.scalar.dma_start(out=S128[32 * qd:32 * qd + 32, :],
                                            in_=S128[0:32, :])
                elif c == 1:
                    nc.scalar.dma_start(
                        out=wo_sb,
                        in_=bass.AP(tensor=Wo, offset=0,
                                    ap=[[D, 128], [D * 128, 2], [1, D]]))
                else:
                    for xh in range(2):
                        nc.sync.dma_start(
                            out=xt[:, 8 * xh:8 * xh + 8, :],
                            in_=bass.AP(tensor=xT,
                                        offset=xh * 8 * 128 * T + lo,
                                        ap=[[T, 128], [T * 128, 8], [1, TC]]))
                if c <= 3:
                    # qpos pieces 2c, 2c+1 (needed by fused masks from chunk c)
                    for qq in range(2):
                        qlo = (2 * c + qq) * TC
                        nc.scalar.dma_start(
                            out=qpos_all[:, qlo:qlo + TC],
                            in_=bass.AP(tensor=qpos, offset=qlo,
                                        ap=[[0, 128], [1, TC]]))
                psA = ps_proj.tile([128, TC], f32, tag="psA")
                psB = ps_proj.tile([128, TC], f32, tag="psB")
                psC = ps_proj.tile([128, TC], f32, tag="psC")
                for kk in range(16):
                    st, sp = kk == 0, kk == 15
                    w = wall_sb[:, kk, :]
                    xk = xt[:, kk, :]
                    nc.tensor.matmul(out=psA, lhsT=w[:, 0:128], rhs=xk,
                                     start=st, stop=sp)
                    nc.tensor.matmul(out=psB, lhsT=w[:, 128:256], rhs=xk,
                                     start=st, stop=sp)
                    nc.tensor.matmul(out=psC, lhsT=w[:, 256:384], rhs=xk,
                                     start=st, stop=sp)
                    if pend:
                        pend.pop(0)()
                    if c == 7 and pend:
                        pend.pop(0)()

                # q_sem copies into the 64-row head slots (ACT, bf16 out)
                nc.scalar.copy(out=q01_all[0:32, sl], in_=psA[0:32, :])
                nc.scalar.copy(out=q01_all[64:96, sl], in_=psA[32:64, :])
                nc.scalar.copy(out=q23_all[0:32, sl], in_=psA[64:96, :])
                nc.scalar.copy(out=q23_all[64:96, sl], in_=psA[96:128, :])
                # q_geo RoPE for all 4 heads on DVE
                swq = tmp.tile([128, TC], f32, tag="swq")
                t1q = tmp.tile([128, TC], bf16, tag="t1q")
                sw2 = tmp.tile([128, TC], bf16, tag="sw2")
                nc.vector.stream_shuffle(out=swq, in_=psB, mask=SWAP16)
                nc.vector.tensor_mul(t1q, psB, C128[:, sl])
                nc.vector.tensor_mul(sw2, swq, S128[:, sl])
                for h in range(4):
                    nc.vector.tensor_add(
                        qsrc[h][qb4[h] + 32:qb4[h] + 64, sl],
                        t1q[32 * h:32 * h + 32, :], sw2[32 * h:32 * h + 32, :])

                # k side: rope geo, then pool (c<=5) or copy local (c>=6)
                swp = tmp.tile([64, TC], f32, tag="swp")
                t1 = tmp.tile([32, TC], bf16, tag="t1")
                t2 = tmp.tile([32, TC], bf16, tag="t2")
                blk = psC[32:64, :]
                nc.vector.stream_shuffle(out=swp[32:64, :], in_=blk, mask=SWAP16)
                nc.vector.tensor_mul(t1, blk, C128[0:32, sl])
                nc.vector.tensor_mul(t2, swp[32:64, :], S128[32:64, sl])
                if c <= 5:
                    ktmp = tmp.tile([64, TC], bf16, tag="ktmp")
                    nc.scalar.copy(out=ktmp[0:32, :], in_=psC[0:32, :])
                    nc.vector.tensor_add(ktmp[32:64, :], t1, t2)
                    bs = slice(c * 8, (c + 1) * 8)
                    with nc.allow_low_precision(reason="bf16 pooled keys"):
                        nc.vector.tensor_reduce(
                            out=kTd[0:64, bs],
                            in_=ktmp.rearrange("p (n w) -> p n w", w=MB),
                            axis=mybir.AxisListType.X, op=Alu.add)
                    nc.vector.tensor_scalar_mul(kTd[0:64, bs], kTd[0:64, bs], 1.0 / MB)
                    nc.vector.tensor_reduce(
                        out=vT[:, bs],
                        in_=psC[64:128, :].rearrange("p (n w) -> p n w", w=MB),
                        axis=mybir.AxisListType.X, op=Alu.add)
                    nc.vector.tensor_scalar_mul(vT[:, bs], vT[:, bs], 1.0 / MB)
                    # incremental kTd dup for the new pooled cols
                    nc.scalar.copy(out=kTd[64:128, bs], in_=kTd[0:64, bs])
                else:
                    loff = 128 + (c - 6) * TC
                    lsl = slice(loff, loff + TC)
                    nc.scalar.copy(out=kTd[0:32, lsl], in_=psC[0:32, :])
                    nc.vector.tensor_add(kTd[32:64, lsl], t1, t2)
                    nc.scalar.copy(out=vT[:, lsl], in_=psC[64:128, :])
                    if c == 6:
                        nc.scalar.copy(out=kTd[64:128, lsl],
                                       in_=kTd[0:64, lsl])

                while pend:
                    pend.pop(0)()
                pend = nextp
                nextp = fused_attn_closures(c) if c <= 5 else []
            while pend:
                pend.pop(0)()

        # ===== TAIL: attention chunks 6,7 + all out-projections =====
        with tc.tile_pool(name="pssc", bufs=2, space="PSUM") as ps_sc, \
             tc.tile_pool(name="psout", bufs=2, space="PSUM") as ps_out, \
             tc.tile_pool(name="psy", bufs=4, space="PSUM") as ps_y:

            # remaining local kTd dup (chunk 7's cols; chunk 6's were done
            # in iteration 6's epilogue)
            nc.scalar.copy(out=kTd[64:128, 640:KPAD], in_=kTd[0:64, 640:KPAD])
            # vsum + uniform-row patch for chunk 0 (queries with no visible key)
            nc.vector.tensor_reduce(out=vsum, in_=vT, axis=mybir.AxisListType.X,
                                    op=Alu.add)
            nc.vector.tensor_scalar_mul(vsum, vsum, 1.0 / float(NKEY))
            for dst in (aT01, aT23):
                for base in (0, 64):
                    nc.vector.tensor_copy(
                        out=dst[base:base + 64, 0:63],
                        in_=vsum.broadcast_to([64, 63]))

            # local V transposes into v2 — issued as early fillers
            def v2_unit(i):
                pv = ps_sc.tile([128, 64], f32, tag="sc", name=f"pv{i}")
                nc.tensor.transpose(out=pv,
                                    in_=vT[:, 128 + 128 * i:256 + 128 * i],
                                    identity=ident_sb)
                nc.vector.tensor_copy(out=v2[:, 1 + i, 0:64], in_=pv)

            ysb = {}
            cast_ctr = [0, 0]   # [chunk-local cast idx, chunk]

            def outproj_unit(tt, nn):
                tsl = slice(tt * 128, (tt + 1) * 128)
                nsl = slice(nn * 512, (nn + 1) * 512)
                yp = ps_y.tile([128, 512], f32, tag="yp")
                nc.tensor.matmul(out=yp, lhsT=aT01[:, tsl],
                                 rhs=wo_sb[:, 0, nsl], start=True, stop=False)
                nc.tensor.matmul(out=yp, lhsT=aT23[:, tsl],
                                 rhs=wo_sb[:, 1, nsl], start=False, stop=True)
                if tt not in ysb:
                    ysb[tt] = ypool.tile([128, D], bf16, tag="ysb",
                                         name=f"ysb{tt}")
                cc = tt // 4
                if cast_ctr[1] != cc:
                    cast_ctr[0], cast_ctr[1] = 0, cc
                on_act = cast_ctr[0] < ACT_CAST_SHARE[cc]
                cast_ctr[0] += 1
                if on_act:
                    nc.scalar.copy(out=ysb[tt][:, nsl], in_=yp)
                else:
                    nc.vector.tensor_copy(out=ysb[tt][:, nsl], in_=yp)
                if nn == 3:
                    nc.sync.dma_start(out=y[tsl, :], in_=ysb.pop(tt))

            fillers = [(lambda i=i: v2_unit(i)) for i in range(8)]
            fillers += [(lambda tt=tt, nn=nn: outproj_unit(tt, nn))
                        for tt in list(range(4, 24)) + list(range(4))
                        for nn in range(4)]
            for c in (6, 7):
                lo = c * TC
                sl = slice(lo, lo + TC)
                tiles, masked = _active_tiles(c)
                mdict = {}
                for (mt, n) in tiles:
                    if mt in masked:
                        m_sb = mpool.tile([128, TC], bf16, tag="mask", bufs=6)
                        nc.vector.tensor_scalar(
                            out=m_sb[0:n, :], in0=qpos_all[0:n, sl],
                            scalar1=kpos_sb[0:n, mt:mt + 1], scalar2=None,
                            op0=Alu.is_ge)
                        mdict[mt] = m_sb

                outp = {}
                last_ti = len(tiles) - 1

                for pair in ((0, 1), (2, 3)):
                    for h in pair:
                        outp[h] = ps_out.tile([65, TC], f32, tag="out",
                                              name=f"outp{c}_{h}")
                    for ti, (kt, n) in enumerate(tiles):
                        ks = slice(kt * 128, kt * 128 + n)
                        es = {}
                        for h in pair:
                            qb = qb4[h]
                            sc = ps_sc.tile([128, TC], f32, tag="sc",
                                            name=f"sc{c}_{h}_{kt}")
                            nc.tensor.matmul(out=sc[0:n, :],
                                             lhsT=kTd[qb:qb + 64, ks],
                                             rhs=qsrc[h][qb:qb + 64, sl],
                                             start=True, stop=True)
                            e_sb = epool.tile([128, TC], bf16, tag="e",
                                              name=f"e{c}_{h}_{kt}")
                            nc.scalar.activation(out=e_sb[0:n, :],
                                                 in_=sc[0:n, :], func=Act.Exp)
                            if kt in mdict:
                                nc.vector.tensor_mul(
                                    e_sb[0:n, :], e_sb[0:n, :],
                                    mdict[kt][0:n, :])
                            es[h] = e_sb
                        for _ in range(4):
                            if fillers:
                                fillers.pop(0)()
                        for h in pair:
                            nc.tensor.matmul(out=outp[h],
                                             lhsT=v2[0:n, kt, :],
                                             rhs=es[h][0:n, :],
                                             start=(ti == 0),
                                             stop=(ti == last_ti))
                    recs = {}
                    for h in pair:
                        rel_unit(c, h, outp[h], recs)
                    norm_pair(c, pair, recs)

                fillers += [(lambda tt=c * 4 + tt, nn=nn: outproj_unit(tt, nn))
                            for tt in range(4) for nn in range(4)]

            while fillers:
                fillers.pop(0)()
    nc.finalize()
    return nc


def _host_inputs(x, Wq_sem, Wk_sem, Wq_geo, Wk_geo, Wv, Wo, logit_scale):
    """Build the 8 per-core input maps."""
    import ml_dtypes
    bf16 = ml_dtypes.bfloat16
    pos = np.arange(T, dtype=np.float64)
    inv = 1.0 / (ROPE_BASE ** (np.arange(0, DG, 2, dtype=np.float64) / DG))
    ang = pos[:, None] * inv[None, :]              # [T, 16]
    cosT = np.cos(ang).T.astype(np.float32)        # [16, T]
    sinT = np.sin(ang).T.astype(np.float32)
    c32 = np.concatenate([cosT, cosT], axis=0)     # [32, T]
    s32 = np.concatenate([-sinT, sinT], axis=0)
    kpos = np.full(KPAD, 1e9, dtype=np.float32)
    kpos[:NBLK] = np.arange(NBLK) * MB + (MB - 1)
    kpos[128:] = np.arange(REMOTE, T)
    qpos = np.arange(T, dtype=np.float32)
    ident = np.eye(64, dtype=np.float32)
    xTs = [np.ascontiguousarray(x[b].T).astype(bf16) for b in range(B)]

    scale = np.exp(logit_scale.astype(np.float64)).astype(np.float32)
    in_maps = []
    for core in range(8):
        b, g = core // 4, core % 4
        W = np.empty((D, 384), dtype=np.float32)
        for h in range(4):
            gh = 4 * g + h
            s = scale[gh] / np.sqrt(np.float32(DS))
            W[:, h * 32:(h + 1) * 32] = Wq_sem[:, gh * DS:(gh + 1) * DS] * s
            W[:, 128 + 32 * h:128 + 32 * h + 16] = \
                Wq_geo[:, gh * DG:gh * DG + 16] * s
            W[:, 128 + 32 * h + 16:128 + 32 * (h + 1)] = \
                Wq_geo[:, gh * DG + 16:(gh + 1) * DG] * s
        W[:, 256:288] = Wk_sem[:, g * DS:(g + 1) * DS]
        W[:, 288:304] = Wk_geo[:, g * DG:g * DG + 16]
        W[:, 304:320] = Wk_geo[:, g * DG + 16:(g + 1) * DG]
        W[:, 320:384] = Wv[:, g * DV:(g + 1) * DV]
        in_maps.append({
            "xT": xTs[b],
            "W_all": W.astype(bf16),
            "Wo": np.ascontiguousarray(Wo[g * 256:(g + 1) * 256, :]).astype(bf16),
            "c32d": c32.astype(bf16), "s32d": s32.astype(bf16),
            "kpos": kpos, "qpos": qpos,
            "ident": ident,
        })
    return in_maps


def kernel(x, Wq_sem, Wk_sem, Wq_geo, Wk_geo, Wv, Wo, logit_scale, _trace=False):
    global _PROG
    import sys
    if "/opt/trn_rl_repo" not in sys.path:
        sys.path.insert(0, "/opt/trn_rl_repo")
    from concourse.bass_utils import run_bass_kernel_spmd

    x = np.asarray(x, dtype=np.float32)
    in_maps = _host_inputs(np.asarray(x, np.float32),
                           np.asarray(Wq_sem, np.float32),
                           np.asarray(Wk_sem, np.float32),
                           np.asarray(Wq_geo, np.float32),
                           np.asarray(Wk_geo, np.float32),
                           np.asarray(Wv, np.float32),
                           np.asarray(Wo, np.float32),
                           np.asarray(logit_scale, np.float32))
    if _PROG is None:
        _PROG = _build_program()
    res = run_bass_kernel_spmd(_PROG, in_maps, list(range(8)), trace=_trace)
    outs = [res.results[i]["y"].astype(np.float32) for i in range(8)]
    out = np.empty((B, T, D), dtype=np.float32)
    for b in range(B):
        out[b] = outs[4 * b] + outs[4 * b + 1] + outs[4 * b + 2] + outs[4 * b + 3]
    if _trace:
        return out, res
    return out


# revision 30
# speedup vs baseline: 1.0776x; 1.0036x over previous
"""Trainium2 Bass kernel for the decoupled sparse-attention layer.

Sharding: 8 cores = 2 batch x 4 GQA head-groups. Core i handles batch
b=i//4 and query heads [4g..4g+4) with KV head g, g=i%4. Each core
computes a partial output y_partial = attn_heads @ Wo_rows(group); the
host sums the 4 group partials per batch element.

Schedule (v3 — fused pipeline):
  Phase 1: projection GEMM stream for all 8 t-chunks + RoPE + incremental
    KV pooling, with the attention for query chunks 0..5 (which only see
    pooled mem-block keys) fused into the stream: attention for chunk c
    is issued as closures popped between the projection matmuls of chunk
    c+2, so its engine work (exp/mask/normalize) hides under the
    PE-dense projection stream. The pooled-V transpose re-transposes the
    whole prefix each chunk (same PE cost; keeps the v2 write at
    partition 0).
  Tail: attention for chunks 6,7 (the key-heavy ones) with all 8 chunks'
    out-projection matmuls interleaved as PE fillers. Softmax
    reciprocals via DVE reciprocal_approx_fast on an SBUF-staged
    denominator row; normalization is fused into the PSUM read with one
    scalar_tensor_tensor per head. PSUM->SBUF output casts alternate
    between ACT and DVE.

Per-core layouts (feature dim on partitions):
  xT      [2048, 4096] bf16 input activations (host pre-transposes)
  W_all   [2048, 384] bf16 fused projection weights, output cols:
            [0:128)   q_sem 4 heads x 32, scaled by exp(ls_h)/sqrt(32)
            [128:256) q_geo head-major [x1(16)|x2(16)] per head, scaled
            [256:288) k_sem 32
            [288:320) k_geo [x1(16)|x2(16)]
            [320:384) v 64
  q01_all/q23_all [128, T] bf16: per 64-row head slot [sem 32|x1' 16|x2' 16]
  Keys: 1152 padded slots = [48 mem-blocks | 80 pad | 1024 local].
"""

import numpy as np

B, T, D = 2, 4096, 2048
H, HKV, DS, DG, DV = 16, 4, 32, 32, 64
MB, LW = 64, 1024
REMOTE = T - LW            # 3072
NBLK = REMOTE // MB        # 48
NKEY = NBLK + LW           # 1072
KPAD = 128 + LW            # 1152 padded key slots
NKT = KPAD // 128          # 9 key tiles
TC = 512                   # t-chunk size
NC_CHUNKS = T // TC        # 8
ROPE_BASE = 10000.0

_PROG = None

SWAP16 = list(range(16, 32)) + list(range(0, 16))

# per-chunk count (of 16 outproj casts) assigned to the scalar engine
ACT_CAST_SHARE = [5, 5, 5, 5, 5, 5, 5, 8]


def _active_tiles(c):
    """Key tiles (tile_idx, nrows) visible to query chunk c, plus which
    tiles need the mask path."""
    tiles = [(0, 8 * (c + 1))] if c <= 5 else [(0, NBLK)]
    if c >= 6:
        nloc = (c - 5) * TC
        for t in range(1, 1 + nloc // 128):
            tiles.append((t, 128))
    masked = set()
    if c <= 5:
        masked.add(0)
    else:
        for t, n in tiles[1:]:
            maxpos = REMOTE + t * 128 - 1
            if maxpos > 512 * c:
                masked.add(t)
    return tiles, masked


def _build_program():
    from contextlib import ExitStack
    import concourse.bass as bass
    import concourse.bacc as bacc
    import concourse.tile as tile
    from concourse import mybir

    f32 = mybir.dt.float32
    bf16 = mybir.dt.bfloat16
    Alu = mybir.AluOpType
    Act = mybir.ActivationFunctionType

    nc = bacc.Bacc()
    xT = nc.declare_dram_parameter("xT", [D, T], bf16, isOutput=False)
    W_all = nc.declare_dram_parameter("W_all", [D, 384], bf16, isOutput=False)
    Wo = nc.declare_dram_parameter("Wo", [256, D], bf16, isOutput=False)
    c32d = nc.declare_dram_parameter("c32d", [32, T], bf16, isOutput=False)
    s32d = nc.declare_dram_parameter("s32d", [32, T], bf16, isOutput=False)
    kpos = nc.declare_dram_parameter("kpos", [KPAD], f32, isOutput=False)
    qpos = nc.declare_dram_parameter("qpos", [T], f32, isOutput=False)
    ident = nc.declare_dram_parameter("ident", [64, 64], f32, isOutput=False)
    y = nc.declare_dram_parameter("y", [T, D], bf16, isOutput=True)

    with tile.TileContext(nc) as tc, ExitStack() as ctx:
        persist = ctx.enter_context(tc.tile_pool(name="persist", bufs=1))
        xpool = ctx.enter_context(tc.tile_pool(name="x", bufs=3))
        tmp = ctx.enter_context(tc.tile_pool(name="tmp", bufs=2))
        epool = ctx.enter_context(tc.tile_pool(name="e", bufs=4))
        mpool = ctx.enter_context(tc.tile_pool(name="m", bufs=4))
        ypool = ctx.enter_context(tc.tile_pool(name="y", bufs=3))
        npool = ctx.enter_context(tc.tile_pool(name="n", bufs=2))

        # ---- persistent SBUF tensors ----
        wall_sb = persist.tile([128, 16, 384], bf16)

        def wall_dma(wh):
            nc.sync.dma_start(
                out=wall_sb[:, 4 * wh:4 * wh + 4, :],
                in_=bass.AP(tensor=W_all, offset=wh * 4 * 128 * 384,
                            ap=[[384, 128], [384 * 128, 4], [1, 384]]))

        wo_sb = persist.tile([128, 2, D], bf16)
        C128 = persist.tile([128, T], bf16)
        S128 = persist.tile([128, T], bf16)
        ident_sb = persist.tile([64, 64], f32)
        nc.scalar.dma_start(out=ident_sb, in_=ident[:, :])
        kpos_sb = persist.tile([128, NKT], f32)
        nc.scalar.dma_start(
            out=kpos_sb,
            in_=bass.AP(tensor=kpos, offset=0, ap=[[1, 128], [128, NKT]]))
        qpos_all = persist.tile([128, T], f32)

        q01_all = persist.tile([128, T], bf16)
        q23_all = persist.tile([128, T], bf16)
        aT01 = persist.tile([128, T], bf16)
        aT23 = persist.tile([128, T], bf16)
        kTd = persist.tile([128, KPAD], bf16)   # [sem32|x1'16|x2'16] dup'd
        vT = persist.tile([64, KPAD], f32)
        nc.vector.memset(vT, 0.0)
        v2 = persist.tile([128, NKT, 65], bf16)  # [key, dv | ones]
        onesrc = persist.tile([128, 1], f32)
        nc.vector.memset(onesrc, 1.0)
        nc.vector.tensor_copy(out=v2[0:NBLK, 0, 64:65], in_=onesrc[0:NBLK, :])
        for t in range(1, NKT):
            nc.vector.tensor_copy(out=v2[:, t, 64:65], in_=onesrc)
        vsum = persist.tile([64, 1], f32)

        qsrc = [q01_all, q01_all, q23_all, q23_all]
        qb4 = [0, 64, 0, 64]          # 64-row slot base per head
        aTs = [aT01, aT01, aT23, aT23]

        def rel_unit(c, h, outp_h, recs):
            """Free an outp bank fast: den stage + unnormalized aT cast on
            ACT, reciprocal on DVE."""
            sl = slice(c * TC, (c + 1) * TC)
            hh = h % 2
            den = npool.tile([1, TC], f32, tag=f"den{hh}", bufs=1,
                             name=f"den_{c}_{h}")
            nc.scalar.copy(out=den, in_=outp_h[64:65, :])
            base = qb4[h]
            nc.scalar.copy(out=aTs[h][base:base + 64, sl],
                           in_=outp_h[0:64, :])
            rec = npool.tile([1, TC], f32, tag=f"rec{hh}", bufs=1,
                             name=f"rec_{c}_{h}")
            nc.vector.reciprocal_approx_fast(out=rec, in_=den)
            recs[h] = rec

        def norm_pair(c, pair, recs):
            """Trailing normalize: assemble the pair's [128,TC] reciprocal
            broadcast (both TT inputs must share base partition 0) and one
            in-place multiply."""
            sl = slice(c * TC, (c + 1) * TC)
            rbP = npool.tile([128, TC], f32, tag="rbP", bufs=2,
                             name=f"rbP_{c}_{pair[0]}")
            rbt = npool.tile([64, TC], f32, tag="rbt", bufs=2,
                             name=f"rbt_{c}_{pair[0]}")
            nc.gpsimd.partition_broadcast(out_ap=rbP[0:64, :],
                                          in_ap=recs[pair[0]])
            nc.gpsimd.partition_broadcast(out_ap=rbt, in_ap=recs[pair[1]])
            nc.vector.tensor_copy(out=rbP[64:128, :], in_=rbt)
            dst = aT01 if pair[0] < 2 else aT23
            nc.vector.tensor_mul(dst[:, sl], dst[:, sl], rbP)

        # ========== PHASE 1: projections + fused attention (chunks 0-5) ====
        with tc.tile_pool(name="psp", bufs=2, space="PSUM") as ps_proj, \
             tc.tile_pool(name="psatt", bufs=1, space="PSUM") as ps_att:

            def fused_attn_closures(cq):
                """Attention for query chunk cq (pooled keys only), issued
                as closures popped between chunk cq+2's projection matmuls."""
                lo = cq * TC
                sl = slice(lo, lo + TC)
                n = 8 * (cq + 1)
                m_sb = mpool.tile([128, TC], bf16, tag="mask", bufs=6,
                                  name=f"mf{cq}")
                nc.vector.tensor_scalar(
                    out=m_sb[0:n, :], in0=qpos_all[0:n, sl],
                    scalar1=kpos_sb[0:n, 0:1], scalar2=None, op0=Alu.is_ge)
                es = {}
                outp = {}

                def vtr():
                    pv = ps_att.tile([128, 64], f32, tag="sc", name=f"pvf{cq}")
                    nc.tensor.transpose(out=pv[0:n, :], in_=vT[:, 0:n],
                                        identity=ident_sb)
                    nc.vector.tensor_copy(out=v2[0:n, 0, 0:64], in_=pv[0:n, :])

                def mk_sc(h):
                    def f():
                        qb = qb4[h]
                        sc = ps_att.tile([128, TC], f32, tag="sc",
                                         name=f"scf{cq}_{h}")
                        nc.tensor.matmul(out=sc[0:n, :],
                                         lhsT=kTd[qb:qb + 64, 0:n],
                                         rhs=qsrc[h][qb:qb + 64, sl],
                                         start=True, stop=True)
                        e_sb = epool.tile([128, TC], bf16, tag="e",
                                          name=f"ef{cq}_{h}")
                        nc.scalar.activation(out=e_sb[0:n, :], in_=sc[0:n, :],
                                             func=Act.Exp)
                        nc.vector.tensor_mul(e_sb[0:n, :], e_sb[0:n, :],
                                             m_sb[0:n, :])
                        es[h] = e_sb
                    return f

                def mk_pv(h):
                    def f():
                        outp[h] = ps_att.tile([65, TC], f32, tag="outp",
                                              name=f"outpf{cq}_{h}")
                        nc.tensor.matmul(out=outp[h], lhsT=v2[0:n, 0, :],
                                         rhs=es[h][0:n, :],
                                         start=True, stop=True)
                    return f

                recs = {}

                def mk_fin(h):
                    return lambda: rel_unit(cq, h, outp[h], recs)

                def mk_norm(pair):
                    return lambda: norm_pair(cq, pair, recs)

                return [vtr, mk_sc(0), mk_pv(0), mk_sc(1), mk_fin(0),
                        mk_pv(1), mk_sc(2), mk_fin(1), mk_norm((0, 1)),
                        mk_pv(2), mk_sc(3), mk_fin(2), mk_pv(3), mk_fin(3),
                        mk_norm((2, 3))]

            pend, nextp = [], []
            xts = {}
            for c in range(NC_CHUNKS):
                lo = c * TC
                sl = slice(lo, lo + TC)
                xt = xts[c] if c == 1 else xpool.tile([128, 16, TC], bf16,
                                                       tag="xt")
                if c == 0:
                    # interleave x and W pieces so the first matmul's inputs
                    # land first (startup is HBM-bandwidth-bound)
                    def xt_dma(kq):
                        nc.sync.dma_start(
                            out=xt[:, 4 * kq:4 * kq + 4, :],
                            in_=bass.AP(tensor=xT, offset=kq * 4 * 128 * T + lo,
                                        ap=[[T, 128], [T * 128, 4], [1, TC]]))
                    xt_dma(0); wall_dma(0); wall_dma(1); xt_dma(1)
                    wall_dma(2); wall_dma(3); xt_dma(2); xt_dma(3)
                    # prefetch chunk 1's x so the c0->c1 boundary has DMA lead
                    xts[1] = xpool.tile([128, 16, TC], bf16, tag="xt",
                                        name="xt1pf")
                    for xh in range(2):
                        nc.sync.dma_start(
                            out=xts[1][:, 8 * xh:8 * xh + 8, :],
                            in_=bass.AP(tensor=xT,
                                        offset=xh * 8 * 128 * T + TC,
                                        ap=[[T, 128], [T * 128, 8], [1, TC]]))
                    # cos/sin: one HBM load + on-chip replication (SB->SB DMA)
                    nc.scalar.dma_start(out=C128[0:32, :], in_=c32d[:, :])
                    nc.scalar.dma_start(out=S128[0:32, :], in_=s32d[:, :])
                    for qd in range(1, 4):
                        nc.scalar.dma_start(out=C128[32 * qd:32 * qd + 32, :],
                                            in_=C128[0:32, :])
                        nc.scalar.dma_start(out=S128[32 * qd:32 * qd + 32, :],
                                            in_=S128[0:32, :])
                elif c == 1:
                    nc.scalar.dma_start(
                        out=wo_sb,
                        in_=bass.AP(tensor=Wo, offset=0,
                                    ap=[[D, 128], [D * 128, 2], [1, D]]))
                else:
                    for xh in range(2):
                        nc.sync.dma_start(
                            out=xt[:, 8 * xh:8 * xh + 8, :],
                            in_=bass.AP(tensor=xT,
                                        offset=xh * 8 * 128 * T + lo,
                                        ap=[[T, 128], [T * 128, 8], [1, TC]]))
                if c <= 3:
                    # qpos pieces 2c, 2c+1 (needed by fused masks from chunk c)
                    for qq in range(2):
                        qlo = (2 * c + qq) * TC
                        nc.scalar.dma_start(
                            out=qpos_all[:, qlo:qlo + TC],
                            in_=bass.AP(tensor=qpos, offset=qlo,
                                        ap=[[0, 128], [1, TC]]))
                psA = ps_proj.tile([128, TC], f32, tag="psA")
                psB = ps_proj.tile([128, TC], f32, tag="psB")
                psC = ps_proj.tile([128, TC], f32, tag="psC")
                for kk in range(16):
                    st, sp = kk == 0, kk == 15
                    w = wall_sb[:, kk, :]
                    xk = xt[:, kk, :]
                    nc.tensor.matmul(out=psA, lhsT=w[:, 0:128], rhs=xk,
                                     start=st, stop=sp)
                    nc.tensor.matmul(out=psB, lhsT=w[:, 128:256], rhs=xk,
                                     start=st, stop=sp)
                    nc.tensor.matmul(out=psC, lhsT=w[:, 256:384], rhs=xk,
                                     start=st, stop=sp)
                    if pend:
                        pend.pop(0)()
                    if c >= 6 and pend:
                        pend.pop(0)()

                # q_sem copies into the 64-row head slots (ACT, bf16 out)
                nc.scalar.copy(out=q01_all[0:32, sl], in_=psA[0:32, :])
                nc.scalar.copy(out=q01_all[64:96, sl], in_=psA[32:64, :])
                nc.scalar.copy(out=q23_all[0:32, sl], in_=psA[64:96, :])
                nc.scalar.copy(out=q23_all[64:96, sl], in_=psA[96:128, :])
                # q_geo RoPE for all 4 heads on DVE
                swq = tmp.tile([128, TC], f32, tag="swq")
                t1q = tmp.tile([128, TC], bf16, tag="t1q")
                sw2 = tmp.tile([128, TC], bf16, tag="sw2")
                nc.vector.stream_shuffle(out=swq, in_=psB, mask=SWAP16)
                nc.vector.tensor_mul(t1q, psB, C128[:, sl])
                nc.vector.tensor_mul(sw2, swq, S128[:, sl])
                for h in range(4):
                    nc.vector.tensor_add(
                        qsrc[h][qb4[h] + 32:qb4[h] + 64, sl],
                        t1q[32 * h:32 * h + 32, :], sw2[32 * h:32 * h + 32, :])

                # k side: rope geo, then pool (c<=5) or copy local (c>=6)
                swp = tmp.tile([64, TC], f32, tag="swp")
                t1 = tmp.tile([32, TC], bf16, tag="t1")
                t2 = tmp.tile([32, TC], bf16, tag="t2")
                blk = psC[32:64, :]
                nc.vector.stream_shuffle(out=swp[32:64, :], in_=blk, mask=SWAP16)
                nc.vector.tensor_mul(t1, blk, C128[0:32, sl])
                nc.vector.tensor_mul(t2, swp[32:64, :], S128[32:64, sl])
                if c <= 5:
                    ktmp = tmp.tile([64, TC], bf16, tag="ktmp")
                    nc.scalar.copy(out=ktmp[0:32, :], in_=psC[0:32, :])
                    nc.vector.tensor_add(ktmp[32:64, :], t1, t2)
                    bs = slice(c * 8, (c + 1) * 8)
                    with nc.allow_low_precision(reason="bf16 pooled keys"):
                        nc.vector.tensor_reduce(
                            out=kTd[0:64, bs],
                            in_=ktmp.rearrange("p (n w) -> p n w", w=MB),
                            axis=mybir.AxisListType.X, op=Alu.add)
                    nc.vector.tensor_scalar_mul(kTd[0:64, bs], kTd[0:64, bs], 1.0 / MB)
                    nc.vector.tensor_reduce(
                        out=vT[:, bs],
                        in_=psC[64:128, :].rearrange("p (n w) -> p n w", w=MB),
                        axis=mybir.AxisListType.X, op=Alu.add)
                    nc.vector.tensor_scalar_mul(vT[:, bs], vT[:, bs], 1.0 / MB)
                    # incremental kTd dup for the new pooled cols
                    nc.scalar.copy(out=kTd[64:128, bs], in_=kTd[0:64, bs])
                else:
                    loff = 128 + (c - 6) * TC
                    lsl = slice(loff, loff + TC)
                    nc.scalar.copy(out=kTd[0:32, lsl], in_=psC[0:32, :])
                    nc.vector.tensor_add(kTd[32:64, lsl], t1, t2)
                    nc.scalar.copy(out=vT[:, lsl], in_=psC[64:128, :])
                    if c == 6:
                        nc.scalar.copy(out=kTd[64:128, lsl],
                                       in_=kTd[0:64, lsl])

                while pend:
                    pend.pop(0)()
                if c == 5:
                    # front-load: chunks 4 AND 5 drain during iteration 6
                    # (chunk-5 data is complete; its closures sit behind
                    # chunk-4's so the early-kk pops never outrun pooling),
                    # leaving iteration 7's engine queues clean for the tail.
                    pend = nextp + fused_attn_closures(5)
                    nextp = []
                else:
                    pend = nextp
                    nextp = fused_attn_closures(c) if c <= 5 else []
            while pend:
                pend.pop(0)()

        # ===== TAIL: attention chunks 6,7 + all out-projections =====
        with tc.tile_pool(name="pssc", bufs=2, space="PSUM") as ps_sc, \
             tc.tile_pool(name="psout", bufs=2, space="PSUM") as ps_out, \
             tc.tile_pool(name="psy", bufs=4, space="PSUM") as ps_y:

            # remaining local kTd dup (chunk 7's cols; chunk 6's were done
            # in iteration 6's epilogue)
            nc.scalar.copy(out=kTd[64:128, 640:KPAD], in_=kTd[0:64, 640:KPAD])
            # vsum + uniform-row patch for chunk 0 (queries with no visible key)
            nc.vector.tensor_reduce(out=vsum, in_=vT, axis=mybir.AxisListType.X,
                                    op=Alu.add)
            nc.vector.tensor_scalar_mul(vsum, vsum, 1.0 / float(NKEY))
            for dst in (aT01, aT23):
                for base in (0, 64):
                    nc.vector.tensor_copy(
                        out=dst[base:base + 64, 0:63],
                        in_=vsum.broadcast_to([64, 63]))

            # local V transposes into v2 — issued as early fillers
            def v2_unit(i):
                pv = ps_sc.tile([128, 64], f32, tag="sc", name=f"pv{i}")
                nc.tensor.transpose(out=pv,
                                    in_=vT[:, 128 + 128 * i:256 + 128 * i],
                                    identity=ident_sb)
                nc.vector.tensor_copy(out=v2[:, 1 + i, 0:64], in_=pv)

            ysb = {}
            cast_ctr = [0, 0]   # [chunk-local cast idx, chunk]

            def outproj_unit(tt, nn):
                tsl = slice(tt * 128, (tt + 1) * 128)
                nsl = slice(nn * 512, (nn + 1) * 512)
                yp = ps_y.tile([128, 512], f32, tag="yp")
                nc.tensor.matmul(out=yp, lhsT=aT01[:, tsl],
                                 rhs=wo_sb[:, 0, nsl], start=True, stop=False)
                nc.tensor.matmul(out=yp, lhsT=aT23[:, tsl],
                                 rhs=wo_sb[:, 1, nsl], start=False, stop=True)
                if tt not in ysb:
                    ysb[tt] = ypool.tile([128, D], bf16, tag="ysb",
                                         name=f"ysb{tt}")
                cc = tt // 4
                if cast_ctr[1] != cc:
                    cast_ctr[0], cast_ctr[1] = 0, cc
                on_act = cast_ctr[0] < ACT_CAST_SHARE[cc]
                cast_ctr[0] += 1
                if on_act:
                    nc.scalar.copy(out=ysb[tt][:, nsl], in_=yp)
                else:
                    nc.vector.tensor_copy(out=ysb[tt][:, nsl], in_=yp)
                if nn == 3:
                    nc.sync.dma_start(out=y[tsl, :], in_=ysb.pop(tt))

            fillers = [(lambda i=i: v2_unit(i)) for i in range(8)]
            fillers += [(lambda tt=tt, nn=nn: outproj_unit(tt, nn))
                        for tt in list(range(4, 24)) + list(range(4))
                        for nn in range(4)]
            for c in (6, 7):
                lo = c * TC
                sl = slice(lo, lo + TC)
                tiles, masked = _active_tiles(c)
                mdict = {}
                for (mt, n) in tiles:
                    if mt in masked:
                        m_sb = mpool.tile([128, TC], bf16, tag="mask", bufs=6)
                        nc.vector.tensor_scalar(
                            out=m_sb[0:n, :], in0=qpos_all[0:n, sl],
                            scalar1=kpos_sb[0:n, mt:mt + 1], scalar2=None,
                            op0=Alu.is_ge)
                        mdict[mt] = m_sb

                outp = {}
                last_ti = len(tiles) - 1

                for pair in ((0, 1), (2, 3)):
                    for h in pair:
                        outp[h] = ps_out.tile([65, TC], f32, tag="out",
                                              name=f"outp{c}_{h}")
                    for ti, (kt, n) in enumerate(tiles):
                        ks = slice(kt * 128, kt * 128 + n)
                        es = {}
                        for h in pair:
                            qb = qb4[h]
                            sc = ps_sc.tile([128, TC], f32, tag="sc",
                                            name=f"sc{c}_{h}_{kt}")
                            nc.tensor.matmul(out=sc[0:n, :],
                                             lhsT=kTd[qb:qb + 64, ks],
                                             rhs=qsrc[h][qb:qb + 64, sl],
                                             start=True, stop=True)
                            e_sb = epool.tile([128, TC], bf16, tag="e",
                                              name=f"e{c}_{h}_{kt}")
                            nc.scalar.activation(out=e_sb[0:n, :],
                                                 in_=sc[0:n, :], func=Act.Exp)
                            if kt in mdict:
                                nc.vector.tensor_mul(
                                    e_sb[0:n, :], e_sb[0:n, :],
                                    mdict[kt][0:n, :])
                            es[h] = e_sb
                        for _ in range(4):
                            if fillers:
                                fillers.pop(0)()
                        for h in pair:
                            nc.tensor.matmul(out=outp[h],
                                             lhsT=v2[0:n, kt, :],
                                             rhs=es[h][0:n, :],
                                             start=(ti == 0),
                                             stop=(ti == last_ti))
                    recs = {}
                    for h in pair:
                        rel_unit(c, h, outp[h], recs)
                    norm_pair(c, pair, recs)

                fillers += [(lambda tt=c * 4 + tt, nn=nn: outproj_unit(tt, nn))
                            for tt in range(4) for nn in range(4)]

            while fillers:
                fillers.pop(0)()
    nc.finalize()
    return nc


def _host_inputs(x, Wq_sem, Wk_sem, Wq_geo, Wk_geo, Wv, Wo, logit_scale):
    """Build the 8 per-core input maps."""
    import ml_dtypes
    bf16 = ml_dtypes.bfloat16
    pos = np.arange(T, dtype=np.float64)
    inv = 1.0 / (ROPE_BASE ** (np.arange(0, DG, 2, dtype=np.float64) / DG))
    ang = pos[:, None] * inv[None, :]              # [T, 16]
    cosT = np.cos(ang).T.astype(np.float32)        # [16, T]
    sinT = np.sin(ang).T.astype(np.float32)
    c32 = np.concatenate([cosT, cosT], axis=0)     # [32, T]
    s32 = np.concatenate([-sinT, sinT], axis=0)
    kpos = np.full(KPAD, 1e9, dtype=np.float32)
    kpos[:NBLK] = np.arange(NBLK) * MB + (MB - 1)
    kpos[128:] = np.arange(REMOTE, T)
    qpos = np.arange(T, dtype=np.float32)
    ident = np.eye(64, dtype=np.float32)
    xTs = [np.ascontiguousarray(x[b].T).astype(bf16) for b in range(B)]

    scale = np.exp(logit_scale.astype(np.float64)).astype(np.float32)
    in_maps = []
    for core in range(8):
        b, g = core // 4, core % 4
        W = np.empty((D, 384), dtype=np.float32)
        for h in range(4):
            gh = 4 * g + h
            s = scale[gh] / np.sqrt(np.float32(DS))
            W[:, h * 32:(h + 1) * 32] = Wq_sem[:, gh * DS:(gh + 1) * DS] * s
            W[:, 128 + 32 * h:128 + 32 * h + 16] = \
                Wq_geo[:, gh * DG:gh * DG + 16] * s
            W[:, 128 + 32 * h + 16:128 + 32 * (h + 1)] = \
                Wq_geo[:, gh * DG + 16:(gh + 1) * DG] * s
        W[:, 256:288] = Wk_sem[:, g * DS:(g + 1) * DS]
        W[:, 288:304] = Wk_geo[:, g * DG:g * DG + 16]
        W[:, 304:320] = Wk_geo[:, g * DG + 16:(g + 1) * DG]
        W[:, 320:384] = Wv[:, g * DV:(g + 1) * DV]
        in_maps.append({
            "xT": xTs[b],
            "W_all": W.astype(bf16),
            "Wo": np.ascontiguousarray(Wo[g * 256:(g + 1) * 256, :]).astype(bf16),
            "c32d": c32.astype(bf16), "s32d": s32.astype(bf16),
            "kpos": kpos, "qpos": qpos,
            "ident": ident,
        })
    return in_maps


def kernel(x, Wq_sem, Wk_sem, Wq_geo, Wk_geo, Wv, Wo, logit_scale, _trace=False):
    global _PROG
    import sys
    if "/opt/trn_rl_repo" not in sys.path:
        sys.path.insert(0, "/opt/trn_rl_repo")
    from concourse.bass_utils import run_bass_kernel_spmd

    x = np.asarray(x, dtype=np.float32)
    in_maps = _host_inputs(np.asarray(x, np.float32),
                           np.asarray(Wq_sem, np.float32),
                           np.asarray(Wk_sem, np.float32),
                           np.asarray(Wq_geo, np.float32),
                           np.asarray(Wk_geo, np.float32),
                           np.asarray(Wv, np.float32),
                           np.asarray(Wo, np.float32),
                           np.asarray(logit_scale, np.float32))
    if _PROG is None:
        _PROG = _build_program()
    res = run_bass_kernel_spmd(_PROG, in_maps, list(range(8)), trace=_trace)
    outs = [res.results[i]["y"].astype(np.float32) for i in range(8)]
    out = np.empty((B, T, D), dtype=np.float32)
    for b in range(B):
        out[b] = outs[4 * b] + outs[4 * b + 1] + outs[4 * b + 2] + outs[4 * b + 3]
    if _trace:
        return out, res
    return out


# revision 32
# speedup vs baseline: 1.0809x; 1.0031x over previous
"""Trainium2 Bass kernel for the decoupled sparse-attention layer.

Sharding: 8 cores = 2 batch x 4 GQA head-groups. Core i handles batch
b=i//4 and query heads [4g..4g+4) with KV head g, g=i%4. Each core
computes a partial output y_partial = attn_heads @ Wo_rows(group); the
host sums the 4 group partials per batch element.

Schedule (v3 — fused pipeline):
  Phase 1: projection GEMM stream for all 8 t-chunks + RoPE + incremental
    KV pooling, with the attention for query chunks 0..5 (which only see
    pooled mem-block keys) fused into the stream: attention for chunk c
    is issued as closures popped between the projection matmuls of chunk
    c+2, so its engine work (exp/mask/normalize) hides under the
    PE-dense projection stream. The pooled-V transpose re-transposes the
    whole prefix each chunk (same PE cost; keeps the v2 write at
    partition 0).
  Tail: attention for chunks 6,7 (the key-heavy ones) with all 8 chunks'
    out-projection matmuls interleaved as PE fillers. Softmax
    reciprocals via DVE reciprocal_approx_fast on an SBUF-staged
    denominator row; normalization is fused into the PSUM read with one
    scalar_tensor_tensor per head. PSUM->SBUF output casts alternate
    between ACT and DVE.

Per-core layouts (feature dim on partitions):
  xT      [2048, 4096] bf16 input activations (host pre-transposes)
  W_all   [2048, 384] bf16 fused projection weights, output cols:
            [0:128)   q_sem 4 heads x 32, scaled by exp(ls_h)/sqrt(32)
            [128:256) q_geo head-major [x1(16)|x2(16)] per head, scaled
            [256:288) k_sem 32
            [288:320) k_geo [x1(16)|x2(16)]
            [320:384) v 64
  q01_all/q23_all [128, T] bf16: per 64-row head slot [sem 32|x1' 16|x2' 16]
  Keys: 1152 padded slots = [48 mem-blocks | 80 pad | 1024 local].
"""

import numpy as np

B, T, D = 2, 4096, 2048
H, HKV, DS, DG, DV = 16, 4, 32, 32, 64
MB, LW = 64, 1024
REMOTE = T - LW            # 3072
NBLK = REMOTE // MB        # 48
NKEY = NBLK + LW           # 1072
KPAD = 128 + LW            # 1152 padded key slots
NKT = KPAD // 128          # 9 key tiles
TC = 512                   # t-chunk size
NC_CHUNKS = T // TC        # 8
ROPE_BASE = 10000.0

_PROG = None

SWAP16 = list(range(16, 32)) + list(range(0, 16))

# per-chunk count (of 16 outproj casts) assigned to the scalar engine
ACT_CAST_SHARE = [5, 5, 5, 5, 5, 5, 5, 8]


def _active_tiles(c):
    """Key tiles (tile_idx, nrows) visible to query chunk c, plus which
    tiles need the mask path."""
    tiles = [(0, 8 * (c + 1))] if c <= 5 else [(0, NBLK)]
    if c >= 6:
        nloc = (c - 5) * TC
        for t in range(1, 1 + nloc // 128):
            tiles.append((t, 128))
    masked = set()
    if c <= 5:
        masked.add(0)
    else:
        for t, n in tiles[1:]:
            maxpos = REMOTE + t * 128 - 1
            if maxpos > 512 * c:
                masked.add(t)
    return tiles, masked


def _build_program():
    from contextlib import ExitStack
    import concourse.bass as bass
    import concourse.bacc as bacc
    import concourse.tile as tile
    from concourse import mybir

    f32 = mybir.dt.float32
    bf16 = mybir.dt.bfloat16
    Alu = mybir.AluOpType
    Act = mybir.ActivationFunctionType

    nc = bacc.Bacc()
    xT = nc.declare_dram_parameter("xT", [D, T], bf16, isOutput=False)
    W_all = nc.declare_dram_parameter("W_all", [D, 384], bf16, isOutput=False)
    Wo = nc.declare_dram_parameter("Wo", [256, D], bf16, isOutput=False)
    c32d = nc.declare_dram_parameter("c32d", [32, T], bf16, isOutput=False)
    s32d = nc.declare_dram_parameter("s32d", [32, T], bf16, isOutput=False)
    kpos = nc.declare_dram_parameter("kpos", [KPAD], f32, isOutput=False)
    qpos = nc.declare_dram_parameter("qpos", [T], f32, isOutput=False)
    ident = nc.declare_dram_parameter("ident", [64, 64], f32, isOutput=False)
    y = nc.declare_dram_parameter("y", [T, D], bf16, isOutput=True)

    with tile.TileContext(nc) as tc, ExitStack() as ctx:
        persist = ctx.enter_context(tc.tile_pool(name="persist", bufs=1))
        xpool = ctx.enter_context(tc.tile_pool(name="x", bufs=3))
        tmp = ctx.enter_context(tc.tile_pool(name="tmp", bufs=2))
        epool = ctx.enter_context(tc.tile_pool(name="e", bufs=6))
        mpool = ctx.enter_context(tc.tile_pool(name="m", bufs=4))
        ypool = ctx.enter_context(tc.tile_pool(name="y", bufs=3))
        npool = ctx.enter_context(tc.tile_pool(name="n", bufs=2))

        # ---- persistent SBUF tensors ----
        wall_sb = persist.tile([128, 16, 384], bf16)

        def wall_dma(wh):
            nc.sync.dma_start(
                out=wall_sb[:, 4 * wh:4 * wh + 4, :],
                in_=bass.AP(tensor=W_all, offset=wh * 4 * 128 * 384,
                            ap=[[384, 128], [384 * 128, 4], [1, 384]]))

        wo_sb = persist.tile([128, 2, D], bf16)
        C128 = persist.tile([128, T], bf16)
        S128 = persist.tile([128, T], bf16)
        ident_sb = persist.tile([64, 64], f32)
        nc.scalar.dma_start(out=ident_sb, in_=ident[:, :])
        kpos_sb = persist.tile([128, NKT], f32)
        nc.scalar.dma_start(
            out=kpos_sb,
            in_=bass.AP(tensor=kpos, offset=0, ap=[[1, 128], [128, NKT]]))
        qpos_all = persist.tile([128, T], f32)

        q01_all = persist.tile([128, T], bf16)
        q23_all = persist.tile([128, T], bf16)
        aT01 = persist.tile([128, T], bf16)
        aT23 = persist.tile([128, T], bf16)
        kTd = persist.tile([128, KPAD], bf16)   # [sem32|x1'16|x2'16] dup'd
        vT = persist.tile([64, KPAD], f32)
        nc.vector.memset(vT, 0.0)
        v2 = persist.tile([128, NKT, 65], bf16)  # [key, dv | ones]
        onesrc = persist.tile([128, 1], f32)
        nc.vector.memset(onesrc, 1.0)
        nc.vector.tensor_copy(out=v2[0:NBLK, 0, 64:65], in_=onesrc[0:NBLK, :])
        for t in range(1, NKT):
            nc.vector.tensor_copy(out=v2[:, t, 64:65], in_=onesrc)
        vsum = persist.tile([64, 1], f32)

        qsrc = [q01_all, q01_all, q23_all, q23_all]
        qb4 = [0, 64, 0, 64]          # 64-row slot base per head
        aTs = [aT01, aT01, aT23, aT23]

        def rel_unit(c, h, outp_h, recs):
            """Free an outp bank fast: den stage + unnormalized aT cast on
            ACT, reciprocal on DVE."""
            sl = slice(c * TC, (c + 1) * TC)
            hh = h % 2
            den = npool.tile([1, TC], f32, tag=f"den{hh}", bufs=1,
                             name=f"den_{c}_{h}")
            nc.scalar.copy(out=den, in_=outp_h[64:65, :])
            base = qb4[h]
            nc.scalar.copy(out=aTs[h][base:base + 64, sl],
                           in_=outp_h[0:64, :])
            rec = npool.tile([1, TC], f32, tag=f"rec{hh}", bufs=1,
                             name=f"rec_{c}_{h}")
            nc.vector.reciprocal_approx_fast(out=rec, in_=den)
            recs[h] = rec

        def norm_pair(c, pair, recs):
            """Trailing normalize: assemble the pair's [128,TC] reciprocal
            broadcast (both TT inputs must share base partition 0) and one
            in-place multiply."""
            sl = slice(c * TC, (c + 1) * TC)
            rbP = npool.tile([128, TC], f32, tag="rbP", bufs=2,
                             name=f"rbP_{c}_{pair[0]}")
            rbt = npool.tile([64, TC], f32, tag="rbt", bufs=2,
                             name=f"rbt_{c}_{pair[0]}")
            nc.gpsimd.partition_broadcast(out_ap=rbP[0:64, :],
                                          in_ap=recs[pair[0]])
            nc.gpsimd.partition_broadcast(out_ap=rbt, in_ap=recs[pair[1]])
            nc.vector.tensor_copy(out=rbP[64:128, :], in_=rbt)
            dst = aT01 if pair[0] < 2 else aT23
            nc.vector.tensor_mul(dst[:, sl], dst[:, sl], rbP)

        # ========== PHASE 1: projections + fused attention (chunks 0-5) ====
        with tc.tile_pool(name="psp", bufs=2, space="PSUM") as ps_proj, \
             tc.tile_pool(name="psatt", bufs=1, space="PSUM") as ps_att:

            def fused_attn_closures(cq):
                """Attention for query chunk cq (pooled keys only), issued
                as closures popped between chunk cq+2's projection matmuls."""
                lo = cq * TC
                sl = slice(lo, lo + TC)
                n = 8 * (cq + 1)
                m_sb = mpool.tile([128, TC], bf16, tag="mask", bufs=6,
                                  name=f"mf{cq}")
                nc.vector.tensor_scalar(
                    out=m_sb[0:n, :], in0=qpos_all[0:n, sl],
                    scalar1=kpos_sb[0:n, 0:1], scalar2=None, op0=Alu.is_ge)
                es = {}
                outp = {}

                def vtr():
                    pv = ps_att.tile([128, 64], f32, tag="sc", name=f"pvf{cq}")
                    nc.tensor.transpose(out=pv[0:n, :], in_=vT[:, 0:n],
                                        identity=ident_sb)
                    nc.vector.tensor_copy(out=v2[0:n, 0, 0:64], in_=pv[0:n, :])

                def mk_sc(h):
                    def f():
                        qb = qb4[h]
                        sc = ps_att.tile([128, TC], f32, tag="sc",
                                         name=f"scf{cq}_{h}")
                        nc.tensor.matmul(out=sc[0:n, :],
                                         lhsT=kTd[qb:qb + 64, 0:n],
                                         rhs=qsrc[h][qb:qb + 64, sl],
                                         start=True, stop=True)
                        e_sb = epool.tile([128, TC], bf16, tag="e",
                                          name=f"ef{cq}_{h}")
                        nc.scalar.activation(out=e_sb[0:n, :], in_=sc[0:n, :],
                                             func=Act.Exp)
                        nc.vector.tensor_mul(e_sb[0:n, :], e_sb[0:n, :],
                                             m_sb[0:n, :])
                        es[h] = e_sb
                    return f

                def mk_pv(h):
                    def f():
                        outp[h] = ps_att.tile([65, TC], f32, tag="outp",
                                              name=f"outpf{cq}_{h}")
                        nc.tensor.matmul(out=outp[h], lhsT=v2[0:n, 0, :],
                                         rhs=es[h][0:n, :],
                                         start=True, stop=True)
                    return f

                recs = {}

                def mk_fin(h):
                    return lambda: rel_unit(cq, h, outp[h], recs)

                def mk_norm(pair):
                    return lambda: norm_pair(cq, pair, recs)

                return [vtr, mk_sc(0), mk_pv(0), mk_sc(1), mk_fin(0),
                        mk_pv(1), mk_sc(2), mk_fin(1), mk_norm((0, 1)),
                        mk_pv(2), mk_sc(3), mk_fin(2), mk_pv(3), mk_fin(3),
                        mk_norm((2, 3))]

            pend, nextp = [], []
            xts = {}
            for c in range(NC_CHUNKS):
                lo = c * TC
                sl = slice(lo, lo + TC)
                xt = xts[c] if c == 1 else xpool.tile([128, 16, TC], bf16,
                                                       tag="xt")
                if c == 0:
                    # interleave x and W pieces so the first matmul's inputs
                    # land first (startup is HBM-bandwidth-bound)
                    def xt_dma(kq):
                        nc.sync.dma_start(
                            out=xt[:, 4 * kq:4 * kq + 4, :],
                            in_=bass.AP(tensor=xT, offset=kq * 4 * 128 * T + lo,
                                        ap=[[T, 128], [T * 128, 4], [1, TC]]))
                    xt_dma(0); wall_dma(0); wall_dma(1); xt_dma(1)
                    wall_dma(2); wall_dma(3); xt_dma(2); xt_dma(3)
                    # prefetch chunk 1's x so the c0->c1 boundary has DMA lead
                    xts[1] = xpool.tile([128, 16, TC], bf16, tag="xt",
                                        name="xt1pf")
                    for xh in range(2):
                        nc.sync.dma_start(
                            out=xts[1][:, 8 * xh:8 * xh + 8, :],
                            in_=bass.AP(tensor=xT,
                                        offset=xh * 8 * 128 * T + TC,
                                        ap=[[T, 128], [T * 128, 8], [1, TC]]))
                    # cos/sin: one HBM load + on-chip replication (SB->SB DMA)
                    nc.scalar.dma_start(out=C128[0:32, :], in_=c32d[:, :])
                    nc.scalar.dma_start(out=S128[0:32, :], in_=s32d[:, :])
                    for qd in range(1, 4):
                        nc.scalar.dma_start(out=C128[32 * qd:32 * qd + 32, :],
                                            in_=C128[0:32, :])
                        nc.scalar.dma_start(out=S128[32 * qd:32 * qd + 32, :],
                                            in_=S128[0:32, :])
                elif c == 1:
                    nc.scalar.dma_start(
                        out=wo_sb,
                        in_=bass.AP(tensor=Wo, offset=0,
                                    ap=[[D, 128], [D * 128, 2], [1, D]]))
                else:
                    for xh in range(2):
                        nc.sync.dma_start(
                            out=xt[:, 8 * xh:8 * xh + 8, :],
                            in_=bass.AP(tensor=xT,
                                        offset=xh * 8 * 128 * T + lo,
                                        ap=[[T, 128], [T * 128, 8], [1, TC]]))
                if c <= 3:
                    # qpos pieces 2c, 2c+1 (needed by fused masks from chunk c)
                    for qq in range(2):
                        qlo = (2 * c + qq) * TC
                        nc.scalar.dma_start(
                            out=qpos_all[:, qlo:qlo + TC],
                            in_=bass.AP(tensor=qpos, offset=qlo,
                                        ap=[[0, 128], [1, TC]]))
                psA = ps_proj.tile([128, TC], f32, tag="psA")
                psB = ps_proj.tile([128, TC], f32, tag="psB")
                psC = ps_proj.tile([128, TC], f32, tag="psC")
                for kk in range(16):
                    st, sp = kk == 0, kk == 15
                    w = wall_sb[:, kk, :]
                    xk = xt[:, kk, :]
                    nc.tensor.matmul(out=psA, lhsT=w[:, 0:128], rhs=xk,
                                     start=st, stop=sp)
                    nc.tensor.matmul(out=psB, lhsT=w[:, 128:256], rhs=xk,
                                     start=st, stop=sp)
                    nc.tensor.matmul(out=psC, lhsT=w[:, 256:384], rhs=xk,
                                     start=st, stop=sp)
                    if pend:
                        pend.pop(0)()
                    if c == 7 and pend:
                        pend.pop(0)()

                # q_sem copies into the 64-row head slots (ACT, bf16 out)
                nc.scalar.copy(out=q01_all[0:32, sl], in_=psA[0:32, :])
                nc.scalar.copy(out=q01_all[64:96, sl], in_=psA[32:64, :])
                nc.scalar.copy(out=q23_all[0:32, sl], in_=psA[64:96, :])
                nc.scalar.copy(out=q23_all[64:96, sl], in_=psA[96:128, :])
                # q_geo RoPE for all 4 heads on DVE
                swq = tmp.tile([128, TC], f32, tag="swq")
                t1q = tmp.tile([128, TC], bf16, tag="t1q")
                sw2 = tmp.tile([128, TC], bf16, tag="sw2")
                nc.vector.stream_shuffle(out=swq, in_=psB, mask=SWAP16)
                nc.vector.tensor_mul(t1q, psB, C128[:, sl])
                nc.vector.tensor_mul(sw2, swq, S128[:, sl])
                for h in range(4):
                    nc.vector.tensor_add(
                        qsrc[h][qb4[h] + 32:qb4[h] + 64, sl],
                        t1q[32 * h:32 * h + 32, :], sw2[32 * h:32 * h + 32, :])

                # k side: rope geo, then pool (c<=5) or copy local (c>=6)
                swp = tmp.tile([64, TC], f32, tag="swp")
                t1 = tmp.tile([32, TC], bf16, tag="t1")
                t2 = tmp.tile([32, TC], bf16, tag="t2")
                blk = psC[32:64, :]
                nc.vector.stream_shuffle(out=swp[32:64, :], in_=blk, mask=SWAP16)
                nc.vector.tensor_mul(t1, blk, C128[0:32, sl])
                nc.vector.tensor_mul(t2, swp[32:64, :], S128[32:64, sl])
                if c <= 5:
                    ktmp = tmp.tile([64, TC], bf16, tag="ktmp")
                    nc.scalar.copy(out=ktmp[0:32, :], in_=psC[0:32, :])
                    nc.vector.tensor_add(ktmp[32:64, :], t1, t2)
                    bs = slice(c * 8, (c + 1) * 8)
                    with nc.allow_low_precision(reason="bf16 pooled keys"):
                        nc.vector.tensor_reduce(
                            out=kTd[0:64, bs],
                            in_=ktmp.rearrange("p (n w) -> p n w", w=MB),
                            axis=mybir.AxisListType.X, op=Alu.add)
                    nc.vector.tensor_scalar_mul(kTd[0:64, bs], kTd[0:64, bs], 1.0 / MB)
                    nc.vector.tensor_reduce(
                        out=vT[:, bs],
                        in_=psC[64:128, :].rearrange("p (n w) -> p n w", w=MB),
                        axis=mybir.AxisListType.X, op=Alu.add)
                    nc.vector.tensor_scalar_mul(vT[:, bs], vT[:, bs], 1.0 / MB)
                    # incremental kTd dup for the new pooled cols
                    nc.scalar.copy(out=kTd[64:128, bs], in_=kTd[0:64, bs])
                else:
                    loff = 128 + (c - 6) * TC
                    lsl = slice(loff, loff + TC)
                    nc.scalar.copy(out=kTd[0:32, lsl], in_=psC[0:32, :])
                    nc.vector.tensor_add(kTd[32:64, lsl], t1, t2)
                    nc.scalar.copy(out=vT[:, lsl], in_=psC[64:128, :])
                    if c == 6:
                        nc.scalar.copy(out=kTd[64:128, lsl],
                                       in_=kTd[0:64, lsl])

                while pend:
                    pend.pop(0)()
                pend = nextp
                nextp = fused_attn_closures(c) if c <= 5 else []
            while pend:
                pend.pop(0)()

        # ===== TAIL: attention chunks 6,7 + all out-projections =====
        with tc.tile_pool(name="pssc", bufs=3, space="PSUM") as ps_sc, \
             tc.tile_pool(name="psout", bufs=2, space="PSUM") as ps_out, \
             tc.tile_pool(name="psy", bufs=3, space="PSUM") as ps_y:

            # remaining local kTd dup (chunk 7's cols; chunk 6's were done
            # in iteration 6's epilogue)
            nc.scalar.copy(out=kTd[64:128, 640:KPAD], in_=kTd[0:64, 640:KPAD])
            # vsum + uniform-row patch for chunk 0 (queries with no visible key)
            nc.vector.tensor_reduce(out=vsum, in_=vT, axis=mybir.AxisListType.X,
                                    op=Alu.add)
            nc.vector.tensor_scalar_mul(vsum, vsum, 1.0 / float(NKEY))
            for dst in (aT01, aT23):
                for base in (0, 64):
                    nc.vector.tensor_copy(
                        out=dst[base:base + 64, 0:63],
                        in_=vsum.broadcast_to([64, 63]))

            # local V transposes into v2 — issued as early fillers
            def v2_unit(i):
                pv = ps_sc.tile([128, 64], f32, tag="sc", name=f"pv{i}")
                nc.tensor.transpose(out=pv,
                                    in_=vT[:, 128 + 128 * i:256 + 128 * i],
                                    identity=ident_sb)
                nc.vector.tensor_copy(out=v2[:, 1 + i, 0:64], in_=pv)

            ysb = {}
            cast_ctr = [0, 0]   # [chunk-local cast idx, chunk]

            def outproj_unit(tt, nn):
                tsl = slice(tt * 128, (tt + 1) * 128)
                nsl = slice(nn * 512, (nn + 1) * 512)
                yp = ps_y.tile([128, 512], f32, tag="yp")
                nc.tensor.matmul(out=yp, lhsT=aT01[:, tsl],
                                 rhs=wo_sb[:, 0, nsl], start=True, stop=False)
                nc.tensor.matmul(out=yp, lhsT=aT23[:, tsl],
                                 rhs=wo_sb[:, 1, nsl], start=False, stop=True)
                if tt not in ysb:
                    ysb[tt] = ypool.tile([128, D], bf16, tag="ysb",
                                         name=f"ysb{tt}")
                cc = tt // 4
                if cast_ctr[1] != cc:
                    cast_ctr[0], cast_ctr[1] = 0, cc
                on_act = cast_ctr[0] < ACT_CAST_SHARE[cc]
                cast_ctr[0] += 1
                if on_act:
                    nc.scalar.copy(out=ysb[tt][:, nsl], in_=yp)
                else:
                    nc.vector.tensor_copy(out=ysb[tt][:, nsl], in_=yp)
                if nn == 3:
                    nc.sync.dma_start(out=y[tsl, :], in_=ysb.pop(tt))

            fillers = [(lambda i=i: v2_unit(i)) for i in range(8)]
            fillers += [(lambda tt=tt, nn=nn: outproj_unit(tt, nn))
                        for tt in list(range(4, 24)) + list(range(4))
                        for nn in range(4)]
            for c in (6, 7):
                lo = c * TC
                sl = slice(lo, lo + TC)
                tiles, masked = _active_tiles(c)
                mdict = {}
                for (mt, n) in tiles:
                    if mt in masked:
                        m_sb = mpool.tile([128, TC], bf16, tag="mask", bufs=6)
                        nc.vector.tensor_scalar(
                            out=m_sb[0:n, :], in0=qpos_all[0:n, sl],
                            scalar1=kpos_sb[0:n, mt:mt + 1], scalar2=None,
                            op0=Alu.is_ge)
                        mdict[mt] = m_sb

                outp = {}
                last_ti = len(tiles) - 1

                for pair in ((0, 1), (2, 3)):
                    for h in pair:
                        outp[h] = ps_out.tile([65, TC], f32, tag="out",
                                              name=f"outp{c}_{h}")
                    for ti, (kt, n) in enumerate(tiles):
                        ks = slice(kt * 128, kt * 128 + n)
                        es = {}
                        for h in pair:
                            qb = qb4[h]
                            sc = ps_sc.tile([128, TC], f32, tag="sc",
                                            name=f"sc{c}_{h}_{kt}")
                            nc.tensor.matmul(out=sc[0:n, :],
                                             lhsT=kTd[qb:qb + 64, ks],
                                             rhs=qsrc[h][qb:qb + 64, sl],
                                             start=True, stop=True)
                            e_sb = epool.tile([128, TC], bf16, tag="e",
                                              name=f"e{c}_{h}_{kt}")
                            nc.scalar.activation(out=e_sb[0:n, :],
                                                 in_=sc[0:n, :], func=Act.Exp)
                            if kt in mdict:
                                nc.vector.tensor_mul(
                                    e_sb[0:n, :], e_sb[0:n, :],
                                    mdict[kt][0:n, :])
                            es[h] = e_sb
                        for _ in range(4):
                            if fillers:
                                fillers.pop(0)()
                        for h in pair:
                            nc.tensor.matmul(out=outp[h],
                                             lhsT=v2[0:n, kt, :],
                                             rhs=es[h][0:n, :],
                                             start=(ti == 0),
                                             stop=(ti == last_ti))
                    recs = {}
                    for h in pair:
                        rel_unit(c, h, outp[h], recs)
                    norm_pair(c, pair, recs)

                fillers += [(lambda tt=c * 4 + tt, nn=nn: outproj_unit(tt, nn))
                            for tt in range(4) for nn in range(4)]

            while fillers:
                fillers.pop(0)()
    nc.finalize()
    return nc


def _host_inputs(x, Wq_sem, Wk_sem, Wq_geo, Wk_geo, Wv, Wo, logit_scale):
    """Build the 8 per-core input maps."""
    import ml_dtypes
    bf16 = ml_dtypes.bfloat16
    pos = np.arange(T, dtype=np.float64)
    inv = 1.0 / (ROPE_BASE ** (np.arange(0, DG, 2, dtype=np.float64) / DG))
    ang = pos[:, None] * inv[None, :]              # [T, 16]
    cosT = np.cos(ang).T.astype(np.float32)        # [16, T]
    sinT = np.sin(ang).T.astype(np.float32)
    c32 = np.concatenate([cosT, cosT], axis=0)     # [32, T]
    s32 = np.concatenate([-sinT, sinT], axis=0)
    kpos = np.full(KPAD, 1e9, dtype=np.float32)
    kpos[:NBLK] = np.arange(NBLK) * MB + (MB - 1)
    kpos[128:] = np.arange(REMOTE, T)
    qpos = np.arange(T, dtype=np.float32)
    ident = np.eye(64, dtype=np.float32)
    xTs = [np.ascontiguousarray(x[b].T).astype(bf16) for b in range(B)]

    scale = np.exp(logit_scale.astype(np.float64)).astype(np.float32)
    in_maps = []
    for core in range(8):
        b, g = core // 4, core % 4
        W = np.empty((D, 384), dtype=np.float32)
        for h in range(4):
            gh = 4 * g + h
            s = scale[gh] / np.sqrt(np.float32(DS))
            W[:, h * 32:(h + 1) * 32] = Wq_sem[:, gh * DS:(gh + 1) * DS] * s
            W[:, 128 + 32 * h:128 + 32 * h + 16] = \
                Wq_geo[:, gh * DG:gh * DG + 16] * s
            W[:, 128 + 32 * h + 16:128 + 32 * (h + 1)] = \
                Wq_geo[:, gh * DG + 16:(gh + 1) * DG] * s
        W[:, 256:288] = Wk_sem[:, g * DS:(g + 1) * DS]
        W[:, 288:304] = Wk_geo[:, g * DG:g * DG + 16]
        W[:, 304:320] = Wk_geo[:, g * DG + 16:(g + 1) * DG]
        W[:, 320:384] = Wv[:, g * DV:(g + 1) * DV]
        in_maps.append({
            "xT": xTs[b],
            "W_all": W.astype(bf16),
            "Wo": np.ascontiguousarray(Wo[g * 256:(g + 1) * 256, :]).astype(bf16),
            "c32d": c32.astype(bf16), "s32d": s32.astype(bf16),
            "kpos": kpos, "qpos": qpos,
            "ident": ident,
        })
    return in_maps


def kernel(x, Wq_sem, Wk_sem, Wq_geo, Wk_geo, Wv, Wo, logit_scale, _trace=False):
    global _PROG
    import sys
    if "/opt/trn_rl_repo" not in sys.path:
        sys.path.insert(0, "/opt/trn_rl_repo")
    from concourse.bass_utils import run_bass_kernel_spmd

    x = np.asarray(x, dtype=np.float32)
    in_maps = _host_inputs(np.asarray(x, np.float32),
                           np.asarray(Wq_sem, np.float32),
                           np.asarray(Wk_sem, np.float32),
                           np.asarray(Wq_geo, np.float32),
                           np.asarray(Wk_geo, np.float32),
                           np.asarray(Wv, np.float32),
                           np.asarray(Wo, np.float32),
                           np.asarray(logit_scale, np.float32))
    if _PROG is None:
        _PROG = _build_program()
    res = run_bass_kernel_spmd(_PROG, in_maps, list(range(8)), trace=_trace)
    outs = [res.results[i]["y"].astype(np.float32) for i in range(8)]
    out = np.empty((B, T, D), dtype=np.float32)
    for b in range(B):
        out[b] = outs[4 * b] + outs[4 * b + 1] + outs[4 * b + 2] + outs[4 * b + 3]
    if _trace:
        return out, res
    return out


# revision 33
# speedup vs baseline: 1.0888x; 1.0072x over previous
"""Trainium2 Bass kernel for the decoupled sparse-attention layer.

Sharding: 8 cores = 2 batch x 4 GQA head-groups. Core i handles batch
b=i//4 and query heads [4g..4g+4) with KV head g, g=i%4. Each core
computes a partial output y_partial = attn_heads @ Wo_rows(group); the
host sums the 4 group partials per batch element.

Schedule (v3 — fused pipeline):
  Phase 1: projection GEMM stream for all 8 t-chunks + RoPE + incremental
    KV pooling, with the attention for query chunks 0..5 (which only see
    pooled mem-block keys) fused into the stream: attention for chunk c
    is issued as closures popped between the projection matmuls of chunk
    c+2, so its engine work (exp/mask/normalize) hides under the
    PE-dense projection stream. The pooled-V transpose re-transposes the
    whole prefix each chunk (same PE cost; keeps the v2 write at
    partition 0).
  Tail: attention for chunks 6,7 (the key-heavy ones) with all 8 chunks'
    out-projection matmuls interleaved as PE fillers. Softmax
    reciprocals via DVE reciprocal_approx_fast on an SBUF-staged
    denominator row; normalization is fused into the PSUM read with one
    scalar_tensor_tensor per head. PSUM->SBUF output casts alternate
    between ACT and DVE.

Per-core layouts (feature dim on partitions):
  xT      [2048, 4096] bf16 input activations (host pre-transposes)
  W_all   [2048, 384] bf16 fused projection weights, output cols:
            [0:128)   q_sem 4 heads x 32, scaled by exp(ls_h)/sqrt(32)
            [128:256) q_geo head-major [x1(16)|x2(16)] per head, scaled
            [256:288) k_sem 32
            [288:320) k_geo [x1(16)|x2(16)]
            [320:384) v 64
  q01_all/q23_all [128, T] bf16: per 64-row head slot [sem 32|x1' 16|x2' 16]
  Keys: 1152 padded slots = [48 mem-blocks | 80 pad | 1024 local].
"""

import numpy as np

B, T, D = 2, 4096, 2048
H, HKV, DS, DG, DV = 16, 4, 32, 32, 64
MB, LW = 64, 1024
REMOTE = T - LW            # 3072
NBLK = REMOTE // MB        # 48
NKEY = NBLK + LW           # 1072
KPAD = 128 + LW            # 1152 padded key slots
NKT = KPAD // 128          # 9 key tiles
TC = 512                   # t-chunk size
NC_CHUNKS = T // TC        # 8
ROPE_BASE = 10000.0

_PROG = None

SWAP16 = list(range(16, 32)) + list(range(0, 16))

# per-chunk count (of 16 outproj casts) assigned to the scalar engine
ACT_CAST_SHARE = [5, 5, 5, 5, 5, 5, 5, 8]


def _active_tiles(c):
    """Key tiles (tile_idx, nrows) visible to query chunk c, plus which
    tiles need the mask path."""
    tiles = [(0, 8 * (c + 1))] if c <= 5 else [(0, NBLK)]
    if c >= 6:
        nloc = (c - 5) * TC
        for t in range(1, 1 + nloc // 128):
            tiles.append((t, 128))
    masked = set()
    if c <= 5:
        masked.add(0)
    else:
        for t, n in tiles[1:]:
            maxpos = REMOTE + t * 128 - 1
            if maxpos > 512 * c:
                masked.add(t)
    return tiles, masked


def _build_program():
    from contextlib import ExitStack
    import concourse.bass as bass
    import concourse.bacc as bacc
    import concourse.tile as tile
    from concourse import mybir

    f32 = mybir.dt.float32
    bf16 = mybir.dt.bfloat16
    Alu = mybir.AluOpType
    Act = mybir.ActivationFunctionType

    nc = bacc.Bacc()
    xT = nc.declare_dram_parameter("xT", [D, T], bf16, isOutput=False)
    W_all = nc.declare_dram_parameter("W_all", [D, 384], bf16, isOutput=False)
    Wo = nc.declare_dram_parameter("Wo", [256, D], bf16, isOutput=False)
    c32d = nc.declare_dram_parameter("c32d", [32, T], bf16, isOutput=False)
    s32d = nc.declare_dram_parameter("s32d", [32, T], bf16, isOutput=False)
    kpos = nc.declare_dram_parameter("kpos", [KPAD], f32, isOutput=False)
    qpos = nc.declare_dram_parameter("qpos", [T], f32, isOutput=False)
    ident = nc.declare_dram_parameter("ident", [64, 64], f32, isOutput=False)
    y = nc.declare_dram_parameter("y", [T, D], bf16, isOutput=True)

    with tile.TileContext(nc) as tc, ExitStack() as ctx:
        persist = ctx.enter_context(tc.tile_pool(name="persist", bufs=1))
        xpool = ctx.enter_context(tc.tile_pool(name="x", bufs=3))
        tmp = ctx.enter_context(tc.tile_pool(name="tmp", bufs=2))
        epool = ctx.enter_context(tc.tile_pool(name="e", bufs=4))
        mpool = ctx.enter_context(tc.tile_pool(name="m", bufs=4))
        ypool = ctx.enter_context(tc.tile_pool(name="y", bufs=3))
        npool = ctx.enter_context(tc.tile_pool(name="n", bufs=2))

        # ---- persistent SBUF tensors ----
        wall_sb = persist.tile([128, 16, 384], bf16)

        def wall_dma(wh):
            nc.sync.dma_start(
                out=wall_sb[:, 4 * wh:4 * wh + 4, :],
                in_=bass.AP(tensor=W_all, offset=wh * 4 * 128 * 384,
                            ap=[[384, 128], [384 * 128, 4], [1, 384]]))

        wo_sb = persist.tile([128, 2, D], bf16)
        C128 = persist.tile([128, T], bf16)
        S128 = persist.tile([128, T], bf16)
        ident_sb = persist.tile([64, 64], f32)
        nc.scalar.dma_start(out=ident_sb, in_=ident[:, :])
        kpos_sb = persist.tile([128, NKT], f32)
        nc.scalar.dma_start(
            out=kpos_sb,
            in_=bass.AP(tensor=kpos, offset=0, ap=[[1, 128], [128, NKT]]))
        qpos_all = persist.tile([128, T], f32)

        q01_all = persist.tile([128, T], bf16)
        q23_all = persist.tile([128, T], bf16)
        aT01 = persist.tile([128, T], bf16)
        aT23 = persist.tile([128, T], bf16)
        kTd = persist.tile([128, KPAD], bf16)   # [sem32|x1'16|x2'16] dup'd
        vT = persist.tile([64, KPAD], f32)
        nc.vector.memset(vT, 0.0)
        v2 = persist.tile([128, NKT, 65], bf16)  # [key, dv | ones]
        onesrc = persist.tile([128, 1], f32)
        nc.vector.memset(onesrc, 1.0)
        nc.vector.tensor_copy(out=v2[0:NBLK, 0, 64:65], in_=onesrc[0:NBLK, :])
        for t in range(1, NKT):
            nc.vector.tensor_copy(out=v2[:, t, 64:65], in_=onesrc)
        vsum = persist.tile([64, 1], f32)

        qsrc = [q01_all, q01_all, q23_all, q23_all]
        qb4 = [0, 64, 0, 64]          # 64-row slot base per head
        aTs = [aT01, aT01, aT23, aT23]

        def rel_unit(c, h, outp_h, recs):
            """Free an outp bank fast: den stage + unnormalized aT cast on
            ACT, reciprocal on DVE."""
            sl = slice(c * TC, (c + 1) * TC)
            hh = h % 2
            den = npool.tile([1, TC], f32, tag=f"den{hh}", bufs=1,
                             name=f"den_{c}_{h}")
            nc.scalar.copy(out=den, in_=outp_h[64:65, :])
            base = qb4[h]
            nc.scalar.copy(out=aTs[h][base:base + 64, sl],
                           in_=outp_h[0:64, :])
            rec = npool.tile([1, TC], f32, tag=f"rec{hh}", bufs=1,
                             name=f"rec_{c}_{h}")
            nc.vector.reciprocal_approx_fast(out=rec, in_=den)
            recs[h] = rec

        def norm_pair(c, pair, recs):
            """Trailing normalize: assemble the pair's [128,TC] reciprocal
            broadcast (both TT inputs must share base partition 0) and one
            in-place multiply."""
            sl = slice(c * TC, (c + 1) * TC)
            rbP = npool.tile([128, TC], f32, tag="rbP", bufs=2,
                             name=f"rbP_{c}_{pair[0]}")
            rbt = npool.tile([64, TC], f32, tag="rbt", bufs=2,
                             name=f"rbt_{c}_{pair[0]}")
            nc.gpsimd.partition_broadcast(out_ap=rbP[0:64, :],
                                          in_ap=recs[pair[0]])
            nc.gpsimd.partition_broadcast(out_ap=rbt, in_ap=recs[pair[1]])
            nc.vector.tensor_copy(out=rbP[64:128, :], in_=rbt)
            dst = aT01 if pair[0] < 2 else aT23
            nc.vector.tensor_mul(dst[:, sl], dst[:, sl], rbP)

        # ========== PHASE 1: projections + fused attention (chunks 0-5) ====
        with tc.tile_pool(name="psp", bufs=2, space="PSUM") as ps_proj, \
             tc.tile_pool(name="psatt", bufs=1, space="PSUM") as ps_att:

            def fused_attn_closures(cq):
                """Attention for query chunk cq (pooled keys only), issued
                as closures popped between chunk cq+2's projection matmuls."""
                lo = cq * TC
                sl = slice(lo, lo + TC)
                n = 8 * (cq + 1)
                m_sb = mpool.tile([128, TC], bf16, tag="mask", bufs=6,
                                  name=f"mf{cq}")
                nc.vector.tensor_scalar(
                    out=m_sb[0:n, :], in0=qpos_all[0:n, sl],
                    scalar1=kpos_sb[0:n, 0:1], scalar2=None, op0=Alu.is_ge)
                es = {}
                outp = {}

                def vtr():
                    pv = ps_att.tile([128, 64], f32, tag="sc", name=f"pvf{cq}")
                    nc.tensor.transpose(out=pv[0:n, :], in_=vT[:, 0:n],
                                        identity=ident_sb)
                    nc.vector.tensor_copy(out=v2[0:n, 0, 0:64], in_=pv[0:n, :])

                def mk_sc(h):
                    def f():
                        qb = qb4[h]
                        sc = ps_att.tile([128, TC], f32, tag="sc",
                                         name=f"scf{cq}_{h}")
                        nc.tensor.matmul(out=sc[0:n, :],
                                         lhsT=kTd[qb:qb + 64, 0:n],
                                         rhs=qsrc[h][qb:qb + 64, sl],
                                         start=True, stop=True)
                        e_sb = epool.tile([128, TC], bf16, tag="e",
                                          name=f"ef{cq}_{h}")
                        nc.scalar.activation(out=e_sb[0:n, :], in_=sc[0:n, :],
                                             func=Act.Exp)
                        nc.vector.tensor_mul(e_sb[0:n, :], e_sb[0:n, :],
                                             m_sb[0:n, :])
                        es[h] = e_sb
                    return f

                def mk_pv(h):
                    def f():
                        outp[h] = ps_att.tile([65, TC], f32, tag="outp",
                                              name=f"outpf{cq}_{h}")
                        nc.tensor.matmul(out=outp[h], lhsT=v2[0:n, 0, :],
                                         rhs=es[h][0:n, :],
                                         start=True, stop=True)
                    return f

                recs = {}

                def mk_fin(h):
                    return lambda: rel_unit(cq, h, outp[h], recs)

                def mk_norm(pair):
                    return lambda: norm_pair(cq, pair, recs)

                return [vtr, mk_sc(0), mk_pv(0), mk_sc(1), mk_fin(0),
                        mk_pv(1), mk_sc(2), mk_fin(1), mk_norm((0, 1)),
                        mk_pv(2), mk_sc(3), mk_fin(2), mk_pv(3), mk_fin(3),
                        mk_norm((2, 3))]

            pend, nextp = [], []
            xts = {}
            for c in range(NC_CHUNKS):
                lo = c * TC
                sl = slice(lo, lo + TC)
                xt = xts[c] if c == 1 else xpool.tile([128, 16, TC], bf16,
                                                       tag="xt")
                if c == 0:
                    # interleave x and W pieces so the first matmul's inputs
                    # land first (startup is HBM-bandwidth-bound)
                    def xt_dma(kq):
                        nc.sync.dma_start(
                            out=xt[:, 4 * kq:4 * kq + 4, :],
                            in_=bass.AP(tensor=xT, offset=kq * 4 * 128 * T + lo,
                                        ap=[[T, 128], [T * 128, 4], [1, TC]]))
                    xt_dma(0); wall_dma(0); wall_dma(1); xt_dma(1)
                    wall_dma(2); wall_dma(3); xt_dma(2); xt_dma(3)
                    # prefetch chunk 1's x so the c0->c1 boundary has DMA lead
                    xts[1] = xpool.tile([128, 16, TC], bf16, tag="xt",
                                        name="xt1pf")
                    for xh in range(2):
                        nc.sync.dma_start(
                            out=xts[1][:, 8 * xh:8 * xh + 8, :],
                            in_=bass.AP(tensor=xT,
                                        offset=xh * 8 * 128 * T + TC,
                                        ap=[[T, 128], [T * 128, 8], [1, TC]]))
                    # cos/sin: one HBM load + on-chip replication (SB->SB DMA)
                    nc.scalar.dma_start(out=C128[0:32, :], in_=c32d[:, :])
                    nc.scalar.dma_start(out=S128[0:32, :], in_=s32d[:, :])
                    for qd in range(1, 4):
                        nc.scalar.dma_start(out=C128[32 * qd:32 * qd + 32, :],
                                            in_=C128[0:32, :])
                        nc.scalar.dma_start(out=S128[32 * qd:32 * qd + 32, :],
                                            in_=S128[0:32, :])
                elif c == 1:
                    nc.scalar.dma_start(
                        out=wo_sb,
                        in_=bass.AP(tensor=Wo, offset=0,
                                    ap=[[D, 128], [D * 128, 2], [1, D]]))
                else:
                    for xh in range(2):
                        nc.sync.dma_start(
                            out=xt[:, 8 * xh:8 * xh + 8, :],
                            in_=bass.AP(tensor=xT,
                                        offset=xh * 8 * 128 * T + lo,
                                        ap=[[T, 128], [T * 128, 8], [1, TC]]))
                if c <= 3:
                    # qpos pieces 2c, 2c+1 (needed by fused masks from chunk c)
                    for qq in range(2):
                        qlo = (2 * c + qq) * TC
                        nc.scalar.dma_start(
                            out=qpos_all[:, qlo:qlo + TC],
                            in_=bass.AP(tensor=qpos, offset=qlo,
                                        ap=[[0, 128], [1, TC]]))
                psA = ps_proj.tile([128, TC], f32, tag="psA")
                psB = ps_proj.tile([128, TC], f32, tag="psB")
                psC = ps_proj.tile([128, TC], f32, tag="psC")
                for kk in range(16):
                    st, sp = kk == 0, kk == 15
                    w = wall_sb[:, kk, :]
                    xk = xt[:, kk, :]
                    nc.tensor.matmul(out=psA, lhsT=w[:, 0:128], rhs=xk,
                                     start=st, stop=sp)
                    nc.tensor.matmul(out=psB, lhsT=w[:, 128:256], rhs=xk,
                                     start=st, stop=sp)
                    nc.tensor.matmul(out=psC, lhsT=w[:, 256:384], rhs=xk,
                                     start=st, stop=sp)
                    if pend:
                        pend.pop(0)()
                    if c == 7 and pend:
                        pend.pop(0)()

                # q_sem copies into the 64-row head slots (ACT, bf16 out)
                nc.scalar.copy(out=q01_all[0:32, sl], in_=psA[0:32, :])
                nc.scalar.copy(out=q01_all[64:96, sl], in_=psA[32:64, :])
                nc.scalar.copy(out=q23_all[0:32, sl], in_=psA[64:96, :])
                nc.scalar.copy(out=q23_all[64:96, sl], in_=psA[96:128, :])
                # q_geo RoPE for all 4 heads on DVE
                swq = tmp.tile([128, TC], f32, tag="swq")
                t1q = tmp.tile([128, TC], bf16, tag="t1q")
                sw2 = tmp.tile([128, TC], bf16, tag="sw2")
                nc.vector.stream_shuffle(out=swq, in_=psB, mask=SWAP16)
                nc.vector.tensor_mul(t1q, psB, C128[:, sl])
                nc.vector.tensor_mul(sw2, swq, S128[:, sl])
                for h in range(4):
                    nc.vector.tensor_add(
                        qsrc[h][qb4[h] + 32:qb4[h] + 64, sl],
                        t1q[32 * h:32 * h + 32, :], sw2[32 * h:32 * h + 32, :])

                # k side: rope geo, then pool (c<=5) or copy local (c>=6)
                swp = tmp.tile([64, TC], f32, tag="swp")
                t1 = tmp.tile([32, TC], bf16, tag="t1")
                t2 = tmp.tile([32, TC], bf16, tag="t2")
                blk = psC[32:64, :]
                nc.vector.stream_shuffle(out=swp[32:64, :], in_=blk, mask=SWAP16)
                nc.vector.tensor_mul(t1, blk, C128[0:32, sl])
                nc.vector.tensor_mul(t2, swp[32:64, :], S128[32:64, sl])
                if c <= 5:
                    ktmp = tmp.tile([64, TC], bf16, tag="ktmp")
                    nc.scalar.copy(out=ktmp[0:32, :], in_=psC[0:32, :])
                    nc.vector.tensor_add(ktmp[32:64, :], t1, t2)
                    bs = slice(c * 8, (c + 1) * 8)
                    with nc.allow_low_precision(reason="bf16 pooled keys"):
                        nc.vector.tensor_reduce(
                            out=kTd[0:64, bs],
                            in_=ktmp.rearrange("p (n w) -> p n w", w=MB),
                            axis=mybir.AxisListType.X, op=Alu.add)
                    nc.vector.tensor_scalar_mul(kTd[0:64, bs], kTd[0:64, bs], 1.0 / MB)
                    nc.vector.tensor_reduce(
                        out=vT[:, bs],
                        in_=psC[64:128, :].rearrange("p (n w) -> p n w", w=MB),
                        axis=mybir.AxisListType.X, op=Alu.add)
                    nc.vector.tensor_scalar_mul(vT[:, bs], vT[:, bs], 1.0 / MB)
                    # incremental kTd dup for the new pooled cols
                    nc.scalar.copy(out=kTd[64:128, bs], in_=kTd[0:64, bs])
                else:
                    loff = 128 + (c - 6) * TC
                    lsl = slice(loff, loff + TC)
                    nc.scalar.copy(out=kTd[0:32, lsl], in_=psC[0:32, :])
                    nc.vector.tensor_add(kTd[32:64, lsl], t1, t2)
                    nc.scalar.copy(out=vT[:, lsl], in_=psC[64:128, :])
                    if c == 6:
                        nc.scalar.copy(out=kTd[64:128, lsl],
                                       in_=kTd[0:64, lsl])

                while pend:
                    pend.pop(0)()
                pend = nextp
                nextp = fused_attn_closures(c) if c <= 5 else []
            while pend:
                pend.pop(0)()

        # ===== TAIL: attention chunks 6,7 + all out-projections =====
        with tc.tile_pool(name="pssc", bufs=2, space="PSUM") as ps_sc, \
             tc.tile_pool(name="psout", bufs=2, space="PSUM") as ps_out, \
             tc.tile_pool(name="psy", bufs=4, space="PSUM") as ps_y:

            # remaining local kTd dup (chunk 7's cols; chunk 6's were done
            # in iteration 6's epilogue)
            nc.scalar.copy(out=kTd[64:128, 640:KPAD], in_=kTd[0:64, 640:KPAD])
            # vsum + uniform-row patch for chunk 0 (queries with no visible key)
            nc.vector.tensor_reduce(out=vsum, in_=vT, axis=mybir.AxisListType.X,
                                    op=Alu.add)
            nc.vector.tensor_scalar_mul(vsum, vsum, 1.0 / float(NKEY))
            for dst in (aT01, aT23):
                for base in (0, 64):
                    nc.vector.tensor_copy(
                        out=dst[base:base + 64, 0:63],
                        in_=vsum.broadcast_to([64, 63]))

            # local V transposes into v2 — issued as early fillers
            def v2_unit(i):
                pv = ps_sc.tile([128, 64], f32, tag="sc", name=f"pv{i}")
                nc.tensor.transpose(out=pv,
                                    in_=vT[:, 128 + 128 * i:256 + 128 * i],
                                    identity=ident_sb)
                nc.vector.tensor_copy(out=v2[:, 1 + i, 0:64], in_=pv)

            ysb = {}
            cast_ctr = [0, 0]   # [chunk-local cast idx, chunk]

            def outproj_unit(tt, nn):
                tsl = slice(tt * 128, (tt + 1) * 128)
                nsl = slice(nn * 512, (nn + 1) * 512)
                yp = ps_y.tile([128, 512], f32, tag="yp")
                nc.tensor.matmul(out=yp, lhsT=aT01[:, tsl],
                                 rhs=wo_sb[:, 0, nsl], start=True, stop=False)
                nc.tensor.matmul(out=yp, lhsT=aT23[:, tsl],
                                 rhs=wo_sb[:, 1, nsl], start=False, stop=True)
                if tt not in ysb:
                    ysb[tt] = ypool.tile([128, D], bf16, tag="ysb",
                                         name=f"ysb{tt}")
                cc = tt // 4
                if cast_ctr[1] != cc:
                    cast_ctr[0], cast_ctr[1] = 0, cc
                on_act = cast_ctr[0] < ACT_CAST_SHARE[cc]
                cast_ctr[0] += 1
                if on_act:
                    nc.scalar.copy(out=ysb[tt][:, nsl], in_=yp)
                else:
                    nc.vector.tensor_copy(out=ysb[tt][:, nsl], in_=yp)
                if nn == 3:
                    nc.sync.dma_start(out=y[tsl, :], in_=ysb.pop(tt))

            fillers = [(lambda i=i: v2_unit(i)) for i in range(8)]
            fillers += [(lambda tt=tt, nn=nn: outproj_unit(tt, nn))
                        for tt in list(range(4, 24)) + list(range(4))
                        for nn in range(4)]
            for c in (6, 7):
                lo = c * TC
                sl = slice(lo, lo + TC)
                tiles, masked = _active_tiles(c)
                mdict = {}
                for (mt, n) in tiles:
                    if mt in masked:
                        m_sb = mpool.tile([128, TC], bf16, tag="mask", bufs=6)
                        nc.vector.tensor_scalar(
                            out=m_sb[0:n, :], in0=qpos_all[0:n, sl],
                            scalar1=kpos_sb[0:n, mt:mt + 1], scalar2=None,
                            op0=Alu.is_ge)
                        mdict[mt] = m_sb

                outp = {}
                last_ti = len(tiles) - 1

                for pair in ((0, 1), (2, 3)):
                    for h in pair:
                        outp[h] = ps_out.tile([65, TC], f32, tag="out",
                                              name=f"outp{c}_{h}")
                    for ti, (kt, n) in enumerate(tiles):
                        ks = slice(kt * 128, kt * 128 + n)
                        es = {}
                        for h in pair:
                            qb = qb4[h]
                            sc = ps_sc.tile([128, TC], f32, tag="sc",
                                            name=f"sc{c}_{h}_{kt}")
                            nc.tensor.matmul(out=sc[0:n, :],
                                             lhsT=kTd[qb:qb + 64, ks],
                                             rhs=qsrc[h][qb:qb + 64, sl],
                                             start=True, stop=True)
                            e_sb = epool.tile([128, TC], bf16, tag="e",
                                              name=f"e{c}_{h}_{kt}")
                            nc.scalar.activation(out=e_sb[0:n, :],
                                                 in_=sc[0:n, :], func=Act.Exp)
                            if kt in mdict:
                                nc.vector.tensor_mul(
                                    e_sb[0:n, :], e_sb[0:n, :],
                                    mdict[kt][0:n, :])
                            es[h] = e_sb
                        for _ in range(4):
                            if fillers:
                                fillers.pop(0)()
                        for h in pair:
                            nc.tensor.matmul(out=outp[h],
                                             lhsT=v2[0:n, kt, :],
                                             rhs=es[h][0:n, :],
                                             start=(ti == 0),
                                             stop=(ti == last_ti))
                    recs = {}
                    for h in pair:
                        rel_unit(c, h, outp[h], recs)
                    norm_pair(c, pair, recs)

                fillers += [(lambda tt=c * 4 + tt, nn=nn: outproj_unit(tt, nn))
                            for tt in range(4) for nn in range(4)]

            while fillers:
                fillers.pop(0)()
    nc.finalize()
    return nc


def _host_inputs(x, Wq_sem, Wk_sem, Wq_geo, Wk_geo, Wv, Wo, logit_scale):
    """Build the 8 per-core input maps."""
    import ml_dtypes
    bf16 = ml_dtypes.bfloat16
    pos = np.arange(T, dtype=np.float64)
    inv = 1.0 / (ROPE_BASE ** (np.arange(0, DG, 2, dtype=np.float64) / DG))
    ang = pos[:, None] * inv[None, :]              # [T, 16]
    cosT = np.cos(ang).T.astype(np.float32)        # [16, T]
    sinT = np.sin(ang).T.astype(np.float32)
    c32 = np.concatenate([cosT, cosT], axis=0)     # [32, T]
    s32 = np.concatenate([-sinT, sinT], axis=0)
    kpos = np.full(KPAD, 1e9, dtype=np.float32)
    kpos[:NBLK] = np.arange(NBLK) * MB + (MB - 1)
    kpos[128:] = np.arange(REMOTE, T)
    qpos = np.arange(T, dtype=np.float32)
    ident = np.eye(64, dtype=np.float32)
    xTs = [np.ascontiguousarray(x[b].T).astype(bf16) for b in range(B)]

    scale = np.exp(logit_scale.astype(np.float64)).astype(np.float32)
    in_maps = []
    for core in range(8):
        b, g = core // 4, core % 4
        W = np.empty((D, 384), dtype=np.float32)
        for h in range(4):
            gh = 4 * g + h
            s = scale[gh] / np.sqrt(np.float32(DS))
            W[:, h * 32:(h + 1) * 32] = Wq_sem[:, gh * DS:(gh + 1) * DS] * s
            W[:, 128 + 32 * h:128 + 32 * h + 16] = \
                Wq_geo[:, gh * DG:gh * DG + 16] * s
            W[:, 128 + 32 * h + 16:128 + 32 * (h + 1)] = \
                Wq_geo[:, gh * DG + 16:(gh + 1) * DG] * s
        W[:, 256:288] = Wk_sem[:, g * DS:(g + 1) * DS]
        W[:, 288:304] = Wk_geo[:, g * DG:g * DG + 16]
        W[:, 304:320] = Wk_geo[:, g * DG + 16:(g + 1) * DG]
        W[:, 320:384] = Wv[:, g * DV:(g + 1) * DV]
        in_maps.append({
            "xT": xTs[b],
            "W_all": W.astype(bf16),
            "Wo": np.ascontiguousarray(Wo[g * 256:(g + 1) * 256, :]).astype(bf16),
            "c32d": c32.astype(bf16), "s32d": s32.astype(bf16),
            "kpos": kpos, "qpos": qpos,
            "ident": ident,
        })
    return in_maps


def kernel(x, Wq_sem, Wk_sem, Wq_geo, Wk_geo, Wv, Wo, logit_scale, _trace=False):
    global _PROG
    import sys
    if "/opt/trn_rl_repo" not in sys.path:
        sys.path.insert(0, "/opt/trn_rl_repo")
    from concourse.bass_utils import run_bass_kernel_spmd

    x = np.asarray(x, dtype=np.float32)
    in_maps = _host_inputs(np.asarray(x, np.float32),
                           np.asarray(Wq_sem, np.float32),
                           np.asarray(Wk_sem, np.float32),
                           np.asarray(Wq_geo, np.float32),
                           np.asarray(Wk_geo, np.float32),
                           np.asarray(Wv, np.float32),
                           np.asarray(Wo, np.float32),
                           np.asarray(logit_scale, np.float32))
    if _PROG is None:
        _PROG = _build_program()
    res = run_bass_kernel_spmd(_PROG, in_maps, list(range(8)), trace=_trace)
    outs = [res.results[i]["y"].astype(np.float32) for i in range(8)]
    out = np.empty((B, T, D), dtype=np.float32)
    for b in range(B):
        out[b] = outs[4 * b] + outs[4 * b + 1] + outs[4 * b + 2] + outs[4 * b + 3]
    if _trace:
        return out, res
    return out
